# revision 49
# baseline (speedup 1.0000x reference)
"""Trainium2 Bass kernel for nn_AttentionBlock2 (gnn_message_passing).

8 NeuronCores, SPMD, no collectives:
  - 2 batches x 4 cores; within a batch, nodes sorted by r-cell and split
    into 4 contiguous cell ranges (disjoint output slices per core).
  - v-gather: dma_gather (SWDGE, 4 queues) of quad-packed bf16 rows
    (4 feature rows per 512B table row -> int16 indexable); on-chip 4-way
    select as one all-bf16 multiply + 3 adds (DVE 2x mode) against a
    scalar-engine-expanded quad mask.
  - Math refactor: q' = (Wq^T Wk / sqrt(E)) r ; output proj Wov = Wo@Wv
    applied after the scatter.
  - One-hot match matrices generated on chip (iota + is_equal) instead of
    streamed from HBM; q'-expansion matmuls accumulate straight into PSUM
    (t-major column layout, no per-t replication copies).
  - scatter-add: PE matmuls of xbar vs one-hot match tiles, PSUM-
    accumulated per 256-cell output window, interleaved with compute.
"""

import sys
import types
import numpy as np
import ml_dtypes

B = 2
CV = 64
CR = 20
E = 64
CO = 64
BUNDLE = 4
P = 128
PER_B = 4
SG = 1024            # nodes per dma_gather call (SG*4 = 4096 idx)
GRP = 512            # nodes per compute group
WCT = 256            # scatter window width (cells)
WCW = 128            # q'-expansion window width (cells)
NEG = -(10 ** 9)

BF16 = ml_dtypes.bfloat16


def _plan(v2p, r2p):
    """Data-dependent but core-uniform plan."""
    Nn = r2p.shape[1]
    cores = []
    for b in range(B):
        cells = r2p[b, :, 0].astype(np.int64)
        order = np.argsort(cells, kind="stable")
        sc = cells[order]
        bounds = []
        for k in range(1, PER_B):
            c = sc[k * Nn // PER_B]
            bounds.append(int(np.searchsorted(sc, c)))
        pb = [0] + bounds + [Nn]
        for pi in range(PER_B):
            lo, hi = pb[pi], pb[pi + 1]
            nodes = order[lo:hi]
            clo = int(sc[lo])
            cores.append(dict(b=b, nodes=nodes, clo=clo,
                              width=int(sc[hi - 1]) + 1 - clo))
    nmax = max(len(c["nodes"]) for c in cores)
    NGRP = -(-nmax // GRP)
    gps = SG // GRP
    if NGRP % gps:
        NGRP += gps - NGRP % gps
    NN = NGRP * GRP
    NCHUNKS = NN // P
    NSGS = NN // SG
    wmax = max(c["width"] for c in cores)
    CT = -(-wmax // WCT)
    W_OUT = CT * WCT
    CTT = W_OUT // WCW

    for c in cores:
        n = len(c["nodes"])
        c["n"] = n
        cr = np.full(NN, NEG, np.int64)
        cr[:n] = r2p[c["b"], c["nodes"], 0].astype(np.int64) - c["clo"]
        c["cell"] = cr
        vr = np.zeros((NN, BUNDLE), np.int64)
        vr[:n] = v2p[c["b"], :, 0].reshape(Nn, BUNDLE)[c["nodes"]]
        c["vrow"] = vr

    ch_lo_s = np.full(CT, 10 ** 9, np.int64)
    ch_hi_s = np.zeros(CT, np.int64)
    ch_lo_t = np.full(CTT, 10 ** 9, np.int64)
    ch_hi_t = np.zeros(CTT, np.int64)
    for c in cores:
        cr = c["cell"]
        valid = cr > NEG
        for W, lo_arr, hi_arr, CN in ((WCT, ch_lo_s, ch_hi_s, CT),
                                      (WCW, ch_lo_t, ch_hi_t, CTT)):
            w_of = np.where(valid, cr // W, -1)
            for wi in range(CN):
                idx = np.nonzero(w_of == wi)[0]
                if len(idx):
                    lo_arr[wi] = min(lo_arr[wi], idx[0] // P)
                    hi_arr[wi] = max(hi_arr[wi], idx[-1] // P + 1)
    ch_lo_s = np.where(ch_lo_s > ch_hi_s, 0, ch_lo_s)
    nwin_s = np.maximum(ch_hi_s - ch_lo_s, 1).astype(np.int64)
    ch_lo_t = np.where(ch_lo_t > ch_hi_t, 0, ch_lo_t)
    nwin_t = np.maximum(ch_hi_t - ch_lo_t, 1).astype(np.int64)

    cover = [[] for _ in range(NCHUNKS)]
    for cw in range(CTT):
        if ch_hi_t[cw] == 0:      # no core has nodes in this window
            continue
        for ch in range(int(ch_lo_t[cw]), int(ch_lo_t[cw] + nwin_t[cw])):
            if 0 <= ch < NCHUNKS:
                cover[ch].append(cw)
    for ch in range(NCHUNKS):
        if not cover[ch]:
            cover[ch].append(0)
        lo, hi = min(cover[ch]), max(cover[ch])
        cover[ch] = list(range(lo, hi + 1))

    KMAX = max(len(cv) for cv in cover)
    ngrp2 = NN // GRP
    KGMAX = max(sum(len(cover[g * 4 + t]) for t in range(4))
                for g in range(ngrp2))
    return dict(cores=cores, NN=NN, NGRP=NGRP, NCHUNKS=NCHUNKS, NSGS=NSGS,
                KGMAX=KGMAX,
                CT=CT, W_OUT=W_OUT, CTT=CTT,
                ch_lo_s=ch_lo_s, nwin_s=nwin_s, cover=cover, KMAX=KMAX)


def _core_arrays(c, plan, v_feat, r_feat, vtab_cache):
    NN, NSGS, CT, W_OUT = plan["NN"], plan["NSGS"], plan["CT"], plan["W_OUT"]
    NCHUNKS = plan["NCHUNKS"]
    b = c["b"]
    out = {}
    if b not in vtab_cache:
        vt = np.ascontiguousarray(v_feat[b].T).astype(BF16)   # [Mv, 64]
        vtab_cache[b] = np.ascontiguousarray(vt.reshape(-1, BUNDLE * CV))
    out["vtab4"] = vtab_cache[b]
    rtT = np.zeros((CR, W_OUT), np.float32)
    w = min(c["width"], W_OUT)
    rtT[:, :w] = r_feat[b][:, c["clo"]: c["clo"] + w]
    out["rtabsT"] = np.ascontiguousarray(rtT).astype(BF16)

    vr = c["vrow"]
    NIDX = SG * BUNDLE
    # position (sg, k*P + p): k = gs*16 + t*4 + j (t-major), node =
    # sg*SG + gs*GRP + t*P + p, bundle member j.
    nodes_all = np.arange(NN)
    gps = SG // GRP
    gs_of = (nodes_all // GRP) % gps
    t_of = (nodes_all % GRP) // P
    p_of = nodes_all % P
    sg_of = nodes_all // SG
    rows = vr                                   # [NN, 4]
    k_of = gs_of * 16 + t_of * 4                # [NN]
    vidx = np.zeros((NSGS, NIDX), np.int64)
    quad = np.zeros((NSGS, P, (SG // P) * 4), np.uint8)
    for j in range(BUNDLE):
        kj = k_of + j
        vidx[sg_of, kj * P + p_of] = rows[:, j] // 4
        quad[sg_of, p_of, kj] = rows[:, j] % 4
    assert vidx.max() < 32768, "v row index exceeds int16 quad range"
    viw = np.tile(vidx.reshape(NSGS, NIDX // 16, 16).transpose(0, 2, 1),
                  (1, 8, 1))                       # [NSGS, P, NIDX//16]
    out["vidx"] = np.ascontiguousarray(
        viw.transpose(1, 0, 2)).astype(np.int16)   # [P, NSGS, NIDX//16]

    qm = np.zeros((NSGS, 4, P, (SG // P) * 4), np.float32)
    for qi in (0, 1, 2, 3):
        qm[:, qi] = (quad == qi)
    out["qmaskf"] = np.ascontiguousarray(
        qm.transpose(2, 0, 1, 3)).astype(BF16)     # [P, NSGS, 4, GPS*16]

    # window-relative cell values for on-chip one-hot generation
    ch_lo_s, nwin_s = plan["ch_lo_s"], plan["nwin_s"]
    KMAX, cover = plan["KMAX"], plan["cover"]
    NWIN = int(nwin_s.sum())
    cell = c["cell"]
    cadj = np.full((P, NWIN), -1, np.int64)
    wi = 0
    for ct in range(CT):
        for wv in range(int(nwin_s[ct])):
            ch = int(ch_lo_s[ct]) + wv
            if ch < NCHUNKS:
                vals = cell[ch * P:(ch + 1) * P] - ct * WCT
                vals = np.where((vals >= 0) & (vals < WCT), vals, -1)
                cadj[:, wi] = vals
            wi += 1
    out["celladjS"] = cadj.astype(BF16)

    NGRP, cover = plan["NGRP"], plan["cover"]
    KGMAX = plan["KGMAX"]
    qmt = np.zeros((NGRP, KGMAX, P, P), np.float32)
    for g in range(NGRP):
        off = 0
        for t in range(4):
            ch = g * 4 + t
            vals = cell[ch * P:(ch + 1) * P]
            for cw in cover[ch]:
                rv = vals - cw * WCW
                ok = (rv >= 0) & (rv < WCW)
                # matchT layout: [cell-rel partition, node]
                qmt[g, off, rv[ok], np.nonzero(ok)[0]] = 1.0
                off += 1
    out["qmats"] = qmt.astype(BF16)
    return out


def _build(plan, Mv):
    import concourse.bacc as bacc
    import concourse.mybir as mybir
    from concourse.tile import TileContext

    NN, NGRP = plan["NN"], plan["NGRP"]
    NCHUNKS, NSGS = plan["NCHUNKS"], plan["NSGS"]
    CT, W_OUT, CTT = plan["CT"], plan["W_OUT"], plan["CTT"]
    ch_lo_s, nwin_s, cover = plan["ch_lo_s"], plan["nwin_s"], plan["cover"]
    KMAX = plan["KMAX"]
    KGMAX = plan["KGMAX"]
    NWIN = int(nwin_s.sum())
    NWMAX = int(nwin_s.max())
    NIDX = SG * BUNDLE
    NCOL = (SG // P) * BUNDLE        # 32 gather columns per supergroup
    TT = SG // P                     # 8 node-subtiles per supergroup

    nc = bacc.Bacc("TRN2", target_bir_lowering=False, debug=False,
                   num_swdge_queues=4)
    dt = mybir.dt
    AL = mybir.AluOpType
    vtab4 = nc.declare_dram_parameter("vtab4", [Mv // 4, BUNDLE * CV], dt.bfloat16, isOutput=False)
    rtabsT_d = nc.declare_dram_parameter("rtabsT", [CR, W_OUT], dt.bfloat16, isOutput=False)
    vidx_d = nc.declare_dram_parameter("vidx", [P, NSGS, NIDX // 16], dt.int16, isOutput=False)
    qmaskf_d = nc.declare_dram_parameter("qmaskf", [P, NSGS, 4, NCOL], dt.bfloat16, isOutput=False)
    celladjS_d = nc.declare_dram_parameter("celladjS", [P, NWIN], dt.bfloat16, isOutput=False)
    qmats_d = nc.declare_dram_parameter("qmats", [NN // GRP, KGMAX, P, P], dt.bfloat16, isOutput=False)
    a16_d = nc.declare_dram_parameter("a16", [CR, E], dt.bfloat16, isOutput=False)
    wov_d = nc.declare_dram_parameter("wovT", [E, CO], dt.bfloat16, isOutput=False)
    out_d = nc.declare_dram_parameter("out", [CO, W_OUT], dt.float32, isOutput=True)

    with TileContext(nc) as tc:
        with (
            tc.tile_pool(name="res", bufs=1) as res,
            tc.tile_pool(name="x4p", bufs=4) as x4p,
            tc.tile_pool(name="mkp", bufs=2) as mkp,
            tc.tile_pool(name="xp", bufs=3) as xp,
            tc.tile_pool(name="small", bufs=2) as small,
            tc.tile_pool(name="scmp", bufs=3) as scmp,
            tc.tile_pool(name="prodp", bufs=3) as prodp,
            tc.tile_pool(name="xbp", bufs=1) as xbp,
            tc.tile_pool(name="scp", bufs=3) as scp,
            tc.tile_pool(name="psB", bufs=3, space="PSUM") as psB,
            tc.tile_pool(name="psC", bufs=2, space="PSUM") as psC,
            tc.tile_pool(name="psD", bufs=2, space="PSUM") as psD,
        ):
            # ---------- resident loads / constants ----------
            vixall = res.tile([P, NSGS, NIDX // 16], dt.int16)
            nc.sync.dma_start(out=vixall[:], in_=vidx_d[:])
            qmaskf = res.tile([P, NSGS, 4, NCOL], dt.bfloat16)
            nc.sync.dma_start(out=qmaskf[:, 0, :, :], in_=qmaskf_d[:, 0])
            nc.sync.dma_start(out=qmaskf[:, 1:, :, :], in_=qmaskf_d[:, 1:])
            celladjS = res.tile([P, NWIN], dt.bfloat16)
            nc.sync.dma_start(out=celladjS[:], in_=celladjS_d[:])
            a16 = res.tile([CR, E], dt.bfloat16)
            nc.sync.dma_start(out=a16[:], in_=a16_d[:])
            wovT = res.tile([E, CO], dt.bfloat16)
            nc.sync.dma_start(out=wovT[:], in_=wov_d[:])
            iotaRow16 = res.tile([P, WCT], dt.int16)
            nc.gpsimd.iota(iotaRow16[:], pattern=[[1, WCT]], base=0,
                           channel_multiplier=0)
            iotaRow = res.tile([P, WCT], dt.bfloat16)
            nc.scalar.copy(out=iotaRow[:], in_=iotaRow16[:])
            qtable = res.tile([P, CTT, E], dt.bfloat16)
            xbar_g = [xbp.tile([P, TT, E], dt.bfloat16, tag=f"xb{g}",
                                 name=f"xbar{g}")
                      for g in range(NSGS)]

            # ---------- q'-table: qtable[cell,:] = rtabsT[:,cell]^T @ A16 ----
            # built lazily, interleaved with the supergroup loop so the
            # bulk build never sits ahead of sg0 on the PE stream
            qtable_built = [0]

            def build_qtable_to(cw_end):
                for cw in range(qtable_built[0], min(cw_end, CTT)):
                    rsl = small.tile([CR, P], dt.bfloat16, tag="rsl")
                    nc.sync.dma_start(out=rsl[:],
                                      in_=rtabsT_d[:, cw * P:(cw + 1) * P])
                    qp = psB.tile([P, 4, E], dt.float32, tag="psB")
                    nc.tensor.matmul(out=qp[:, 0, :], lhsT=rsl[:],
                                     rhs=a16[:], start=True, stop=True)
                    nc.scalar.copy(out=qtable[:, cw, :], in_=qp[:, 0, :])
                qtable_built[0] = max(qtable_built[0], min(cw_end, CTT))

            # ---------- scatter: one-hot gen on chip + PE accumulate ------
            win_start = np.concatenate(([0], np.cumsum(nwin_s)))

            scmm_ready = {}

            def gen_scmm(ct):
                nw = int(nwin_s[ct])
                wi = int(win_start[ct])
                scmm = scmp.tile([P, NWMAX, WCT], dt.bfloat16, tag="scmm",
                                  name=f"scmm{ct}")
                cax = scmp.tile([P, NWMAX, WCT], dt.bfloat16, tag="cax",
                                name=f"cax{ct}")
                nc.scalar.copy(
                    out=cax[:, 0:nw, :],
                    in_=celladjS[:, wi:wi + nw, None].to_broadcast(
                        [P, nw, WCT]))
                nc.vector.tensor_tensor(
                    out=scmm[:, 0:nw, :],
                    in0=cax[:, 0:nw, :],
                    in1=iotaRow[:, None, :].to_broadcast([P, nw, WCT]),
                    op=AL.is_equal)
                scmm_ready[ct] = scmm

            def emit_scatter(ct):
                nw = int(nwin_s[ct])
                if ct not in scmm_ready:
                    gen_scmm(ct)
                scmm = scmm_ready.pop(ct)
                t1 = psC.tile([CO, WCT], dt.float32, tag="psC", name=f"t1_{ct}")
                for wv in range(nw):
                    ch = min(int(ch_lo_s[ct]) + wv, NCHUNKS - 1)
                    nc.tensor.matmul(out=t1[:],
                                     lhsT=xbar_g[ch // TT][:, ch % TT, :],
                                     rhs=scmm[:, wv, :], start=(wv == 0),
                                     stop=(wv == nw - 1))
                t1s = small.tile([CO, WCT], dt.bfloat16, tag="t1s",
                                 name=f"t1s{ct}")
                nc.scalar.copy(out=t1s[:], in_=t1[:])
                ot = psD.tile([CO, WCT], dt.float32, tag="psD", name=f"ot{ct}")
                nc.tensor.matmul(out=ot[:], lhsT=wovT[:], rhs=t1s[:],
                                 start=True, stop=True)
                osb = small.tile([CO, WCT], dt.float32, tag="osb",
                                 name=f"osb{ct}")
                nc.scalar.copy(out=osb[:], in_=ot[:])
                nc.scalar.dma_start(out=out_d[:, ct * WCT:(ct + 1) * WCT],
                                    in_=osb[:])

            # scatter window ct is ready once every chunk it reads is computed
            ct_ready = []
            for ct in range(CT):
                mx = min(int(ch_lo_s[ct]) + int(nwin_s[ct]) - 1, NCHUNKS - 1)
                ct_ready.append(mx + 1)

            issued = {}

            def issue_gather(sg):
                x4 = x4p.tile([P, NCOL, BUNDLE * CV], dt.bfloat16,
                              tag="x4", name=f"x4_{sg}")
                nc.gpsimd.dma_gather(
                    out_ap=x4[:], in_ap=vtab4[:], idxs_ap=vixall[:, sg, :],
                    num_idxs=NIDX, num_idxs_reg=NIDX, elem_size=BUNDLE * CV,
                    single_packet=False, queue_num=sg % 4)
                issued[sg] = x4

            for sg in range(min(4, NSGS)):
                issue_gather(sg)
            mask_tiles = {}
            m0 = mkp.tile([P, NCOL, BUNDLE * CV], dt.bfloat16,
                          tag="mask4", name="mask4_0")
            nc.scalar.copy(
                out=m0[:].rearrange("p k (q c) -> p q k c", q=4),
                in_=qmaskf[:, 0, :, :, None].to_broadcast(
                    [P, 4, NCOL, CV]))
            mask_tiles[0] = m0
            done_ct = 0
            for sg in range(NSGS):
                if sg not in issued:
                    issue_gather(sg)
                x4 = issued.pop(sg)
                need_cw = max(max(cover[ch]) for ch in
                              range(sg * TT, (sg + 1) * TT)) + 1
                build_qtable_to(need_cw + 8)

                # expand NEXT sg's quad mask first so the scalar engine has
                # it ready before that sg's select needs it
                if sg + 1 < NSGS:
                    nmask = mkp.tile([P, NCOL, BUNDLE * CV], dt.bfloat16,
                                     tag="mask4", name=f"mask4_{sg + 1}")
                    nc.scalar.copy(
                        out=nmask[:].rearrange("p k (q c) -> p q k c", q=4),
                        in_=qmaskf[:, sg + 1, :, :, None].to_broadcast(
                            [P, 4, NCOL, CV]))
                    mask_tiles[sg + 1] = nmask
                mask4 = mask_tiles.pop(sg)
                # pre-generate scatter one-hots for windows completing soon
                nd = done_ct
                cd = (sg + 1) * TT
                while nd < CT and ct_ready[nd] <= cd:
                    gen_scmm(nd)
                    nd += 1
                xsel = mask4
                nc.vector.tensor_tensor(out=xsel[:], in0=x4[:], in1=mask4[:],
                                        op=AL.mult)
                xsg = xp.tile([P, NCOL, CV], dt.bfloat16, tag="x",
                              name=f"xsg{sg}")
                nc.vector.tensor_tensor(out=xsg[:], in0=xsel[:, :, 0:CV],
                                        in1=xsel[:, :, CV:2 * CV], op=AL.add)
                nc.vector.tensor_tensor(out=xsg[:], in0=xsg[:],
                                        in1=xsel[:, :, 2 * CV:3 * CV],
                                        op=AL.add)
                nc.vector.tensor_tensor(out=xsg[:], in0=xsg[:],
                                        in1=xsel[:, :, 3 * CV:4 * CV],
                                        op=AL.add)

                ex = scp.tile([P, NCOL], dt.float32, tag="ex")
                for gs in range(2):
                    g = sg * 2 + gs
                    xv = xsg[:, gs * 16:(gs + 1) * 16, :]

                    qps4 = psB.tile([P, 4, E], dt.float32, tag="psB")
                    kg = sum(len(cover[g * 4 + t]) for t in range(4))
                    mtg = small.tile([P, KGMAX, P], dt.bfloat16, tag="mtg")
                    nc.sync.dma_start(
                        out=mtg[:, 0:kg, :],
                        in_=qmats_d[g, 0:kg].rearrange("k p n -> p k n"))
                    off = 0
                    for t in range(4):
                        ch = g * 4 + t
                        cvr = cover[ch]
                        for ci, cw in enumerate(cvr):
                            nc.tensor.matmul(out=qps4[:, t, :],
                                             lhsT=mtg[:, off + ci, :],
                                             rhs=qtable[:, cw, :],
                                             start=(ci == 0),
                                             stop=(ci == len(cvr) - 1))
                        off += len(cvr)
                    qps4b = scp.tile([P, 4, E], dt.bfloat16, tag="qps4b")
                    nc.scalar.copy(out=qps4b[:], in_=qps4[:])
                    # scores: per-point dot(x, q') over channels
                    prod = prodp.tile([P, 16, CV], dt.bfloat16, tag="prod")
                    nc.vector.tensor_tensor(
                        out=prod[:].rearrange("p (t j) c -> p t j c", t=4),
                        in0=xv.rearrange("p (t j) c -> p t j c", t=4),
                        in1=qps4b[:, :, None, :].to_broadcast([P, 4, 4, E]),
                        op=AL.mult)
                    sc = scp.tile([P, 16], dt.bfloat16, tag="sc")
                    with nc.allow_low_precision(reason="bf16 score reduce"):
                        nc.vector.tensor_reduce(out=sc[:], in_=prod[:],
                                                axis=mybir.AxisListType.X,
                                                op=AL.add)
                    nc.scalar.activation(out=ex[:, gs * 16:(gs + 1) * 16],
                                         in_=sc[:],
                                         func=mybir.ActivationFunctionType.Exp)

                # merged softmax tail + weighted sum at supergroup level
                den = scp.tile([P, TT], dt.float32, tag="den")
                nc.vector.tensor_reduce(
                    out=den[:],
                    in_=ex[:].rearrange("p (t j) -> p t j", t=TT),
                    axis=mybir.AxisListType.X, op=AL.add)
                rec = scp.tile([P, TT], dt.float32, tag="rec")
                nc.vector.reciprocal(out=rec[:], in_=den[:])
                attn = scp.tile([P, NCOL], dt.bfloat16, tag="attn")
                nc.vector.tensor_tensor(
                    out=attn[:].rearrange("p (t j) -> p t j", t=TT),
                    in0=ex[:].rearrange("p (t j) -> p t j", t=TT),
                    in1=rec[:, :, None].to_broadcast([P, TT, 4]),
                    op=AL.mult)
                prod2 = scp.tile([P, NCOL, CV], dt.bfloat16, tag="prod2")
                nc.vector.tensor_tensor(
                    out=prod2[:], in0=xsg[:],
                    in1=attn[:, :, None].to_broadcast([P, NCOL, CV]),
                    op=AL.mult)
                pj = prod2[:].rearrange("p (g t j) c -> p j g t c", g=2, t=4)
                xb = xbar_g[sg][:].rearrange("p (g t) c -> p g t c", g=2)
                nc.vector.tensor_tensor(out=xb, in0=pj[:, 0], in1=pj[:, 1],
                                        op=AL.add)
                nc.vector.tensor_tensor(out=xb, in0=xb, in1=pj[:, 2],
                                        op=AL.add)
                nc.vector.tensor_tensor(out=xb, in0=xb, in1=pj[:, 3],
                                        op=AL.add)

                chunks_done = (sg + 1) * TT
                while done_ct < CT and ct_ready[done_ct] <= chunks_done:
                    emit_scatter(done_ct)
                    done_ct += 1

            build_qtable_to(CTT)
            for ct in range(done_ct, CT):
                emit_scatter(ct)
    nc.compile()
    return nc


def _install_ntff_shim():
    try:
        import antenv.axon_hooks  # noqa
        return
    except ImportError:
        pass
    try:
        from trn_agent_boot.trn_boot import _ntff_profile_via_ctypes
        hook = _ntff_profile_via_ctypes('/opt/axon/libaxon_pjrt.so')
        mod = types.ModuleType("antenv.axon_hooks")
        mod.get_axon_ntff_profile_hook = lambda: hook
        mod.set_axon_ntff_profile_hook = lambda h: None
        import antenv
        antenv.axon_hooks = mod
        sys.modules["antenv.axon_hooks"] = mod
    except Exception:
        pass


def kernel(**inputs):
    v_feat = np.asarray(inputs["v_feat"], np.float32)
    r_feat = np.asarray(inputs["r_feat"], np.float32)
    Wq = np.asarray(inputs["Wq"], np.float32)
    Wk = np.asarray(inputs["Wk"], np.float32)
    Wv = np.asarray(inputs["Wv"], np.float32)
    Wo = np.asarray(inputs["Wo"], np.float32)
    v2p = np.asarray(inputs["v2p_ind"])
    r2p = np.asarray(inputs["r2p_ind"])
    Mv = v_feat.shape[2]
    Mr = r_feat.shape[2]

    plan = _plan(v2p, r2p)
    nc = _build(plan, Mv)

    A16 = (Wq.T @ Wk / np.sqrt(np.float32(E))).astype(BF16)
    WovT16 = np.ascontiguousarray((Wo @ Wv).T).astype(BF16)

    in_maps = []
    vtab_cache = {}
    for c in plan["cores"]:
        arr = _core_arrays(c, plan, v_feat, r_feat, vtab_cache)
        arr["a16"] = A16
        arr["wovT"] = WovT16
        in_maps.append(arr)

    from concourse.bass_utils import run_bass_kernel_spmd
    _install_ntff_shim()
    trace = bool(inputs.get("_trace", False))
    res = run_bass_kernel_spmd(nc, in_maps, core_ids=list(range(8)),
                               trace=trace)
    out = np.zeros((B, CO, Mr), np.float32)
    for ci, c in enumerate(plan["cores"]):
        o = res.results[ci]["out"]
        w = min(c["width"], plan["W_OUT"])
        out[c["b"], :, c["clo"]:c["clo"] + w] = o[:, :w]
    kernel.last_exec_time_ns = res.exec_time_ns
    return out


kernel.last_exec_time_ns = None


# revision 50
# speedup vs baseline: 1.0286x; 1.0286x over previous
"""Trainium2 Bass kernel for nn_AttentionBlock2 (gnn_message_passing).

8 NeuronCores, SPMD, no collectives:
  - 2 batches x 4 cores; within a batch, nodes sorted by r-cell and split
    into 4 contiguous cell ranges (disjoint output slices per core).
  - v-gather: dma_gather (SWDGE, 4 queues) of quad-packed bf16 rows
    (4 feature rows per 512B table row -> int16 indexable); on-chip 4-way
    select as one all-bf16 multiply + 3 adds (DVE 2x mode) against a
    scalar-engine-expanded quad mask.
  - Math refactor: q' = (Wq^T Wk / sqrt(E)) r ; output proj Wov = Wo@Wv
    applied after the scatter.
  - One-hot match matrices generated on chip (iota + is_equal) instead of
    streamed from HBM; q'-expansion matmuls accumulate straight into PSUM
    (t-major column layout, no per-t replication copies).
  - scatter-add: PE matmuls of xbar vs one-hot match tiles, PSUM-
    accumulated per 256-cell output window, interleaved with compute.
"""

import sys
import types
import numpy as np
import ml_dtypes

B = 2
CV = 64
CR = 20
E = 64
CO = 64
BUNDLE = 4
P = 128
PER_B = 4
SG = 1024            # nodes per dma_gather call (SG*4 = 4096 idx)
GRP = 512            # nodes per compute group
WCT = 256            # scatter window width (cells)
WCW = 128            # q'-expansion window width (cells)
NEG = -(10 ** 9)

BF16 = ml_dtypes.bfloat16


def _plan(v2p, r2p):
    """Data-dependent but core-uniform plan."""
    Nn = r2p.shape[1]
    cores = []
    for b in range(B):
        cells = r2p[b, :, 0].astype(np.int64)
        order = np.argsort(cells, kind="stable")
        sc = cells[order]
        bounds = []
        for k in range(1, PER_B):
            c = sc[k * Nn // PER_B]
            bounds.append(int(np.searchsorted(sc, c)))
        pb = [0] + bounds + [Nn]
        for pi in range(PER_B):
            lo, hi = pb[pi], pb[pi + 1]
            nodes = order[lo:hi]
            clo = int(sc[lo])
            cores.append(dict(b=b, nodes=nodes, clo=clo,
                              width=int(sc[hi - 1]) + 1 - clo))
    nmax = max(len(c["nodes"]) for c in cores)
    NGRP = -(-nmax // GRP)
    gps = SG // GRP
    if NGRP % gps:
        NGRP += gps - NGRP % gps
    NN = NGRP * GRP
    NCHUNKS = NN // P
    NSGS = NN // SG
    wmax = max(c["width"] for c in cores)
    CT = -(-wmax // WCT)
    W_OUT = CT * WCT
    CTT = W_OUT // WCW

    for c in cores:
        n = len(c["nodes"])
        c["n"] = n
        cr = np.full(NN, NEG, np.int64)
        cr[:n] = r2p[c["b"], c["nodes"], 0].astype(np.int64) - c["clo"]
        c["cell"] = cr
        vr = np.zeros((NN, BUNDLE), np.int64)
        vr[:n] = v2p[c["b"], :, 0].reshape(Nn, BUNDLE)[c["nodes"]]
        c["vrow"] = vr

    ch_lo_s = np.full(CT, 10 ** 9, np.int64)
    ch_hi_s = np.zeros(CT, np.int64)
    ch_lo_t = np.full(CTT, 10 ** 9, np.int64)
    ch_hi_t = np.zeros(CTT, np.int64)
    for c in cores:
        cr = c["cell"]
        valid = cr > NEG
        for W, lo_arr, hi_arr, CN in ((WCT, ch_lo_s, ch_hi_s, CT),
                                      (WCW, ch_lo_t, ch_hi_t, CTT)):
            w_of = np.where(valid, cr // W, -1)
            for wi in range(CN):
                idx = np.nonzero(w_of == wi)[0]
                if len(idx):
                    lo_arr[wi] = min(lo_arr[wi], idx[0] // P)
                    hi_arr[wi] = max(hi_arr[wi], idx[-1] // P + 1)
    ch_lo_s = np.where(ch_lo_s > ch_hi_s, 0, ch_lo_s)
    nwin_s = np.maximum(ch_hi_s - ch_lo_s, 1).astype(np.int64)
    ch_lo_t = np.where(ch_lo_t > ch_hi_t, 0, ch_lo_t)
    nwin_t = np.maximum(ch_hi_t - ch_lo_t, 1).astype(np.int64)

    cover = [[] for _ in range(NCHUNKS)]
    for cw in range(CTT):
        if ch_hi_t[cw] == 0:      # no core has nodes in this window
            continue
        for ch in range(int(ch_lo_t[cw]), int(ch_lo_t[cw] + nwin_t[cw])):
            if 0 <= ch < NCHUNKS:
                cover[ch].append(cw)
    for ch in range(NCHUNKS):
        if not cover[ch]:
            cover[ch].append(0)
        lo, hi = min(cover[ch]), max(cover[ch])
        cover[ch] = list(range(lo, hi + 1))

    KMAX = max(len(cv) for cv in cover)
    ngrp2 = NN // GRP
    KGMAX = max(sum(len(cover[g * 4 + t]) for t in range(4))
                for g in range(ngrp2))
    return dict(cores=cores, NN=NN, NGRP=NGRP, NCHUNKS=NCHUNKS, NSGS=NSGS,
                KGMAX=KGMAX,
                CT=CT, W_OUT=W_OUT, CTT=CTT,
                ch_lo_s=ch_lo_s, nwin_s=nwin_s, cover=cover, KMAX=KMAX)


def _core_arrays(c, plan, v_feat, r_feat, vtab_cache):
    NN, NSGS, CT, W_OUT = plan["NN"], plan["NSGS"], plan["CT"], plan["W_OUT"]
    NCHUNKS = plan["NCHUNKS"]
    b = c["b"]
    out = {}
    if b not in vtab_cache:
        vt = np.ascontiguousarray(v_feat[b].T).astype(BF16)   # [Mv, 64]
        vtab_cache[b] = np.ascontiguousarray(vt.reshape(-1, BUNDLE * CV))
    out["vtab4"] = vtab_cache[b]
    rtT = np.zeros((CR, W_OUT), np.float32)
    w = min(c["width"], W_OUT)
    rtT[:, :w] = r_feat[b][:, c["clo"]: c["clo"] + w]
    out["rtabsT"] = np.ascontiguousarray(rtT).astype(BF16)

    vr = c["vrow"]
    NIDX = SG * BUNDLE
    # position (sg, k*P + p): k = gs*16 + t*4 + j (t-major), node =
    # sg*SG + gs*GRP + t*P + p, bundle member j.
    nodes_all = np.arange(NN)
    gps = SG // GRP
    gs_of = (nodes_all // GRP) % gps
    t_of = (nodes_all % GRP) // P
    p_of = nodes_all % P
    sg_of = nodes_all // SG
    rows = vr                                   # [NN, 4]
    k_of = gs_of * 16 + t_of * 4                # [NN]
    vidx = np.zeros((NSGS, NIDX), np.int64)
    quad = np.zeros((NSGS, P, (SG // P) * 4), np.uint8)
    for j in range(BUNDLE):
        kj = k_of + j
        vidx[sg_of, kj * P + p_of] = rows[:, j] // 4
        quad[sg_of, p_of, kj] = rows[:, j] % 4
    assert vidx.max() < 32768, "v row index exceeds int16 quad range"
    viw = np.tile(vidx.reshape(NSGS, NIDX // 16, 16).transpose(0, 2, 1),
                  (1, 8, 1))                       # [NSGS, P, NIDX//16]
    out["vidx"] = np.ascontiguousarray(
        viw.transpose(1, 0, 2)).astype(np.int16)   # [P, NSGS, NIDX//16]

    qm = np.zeros((NSGS, 4, P, (SG // P) * 4), np.float32)
    for qi in (0, 1, 2, 3):
        qm[:, qi] = (quad == qi)
    out["qmaskf"] = np.ascontiguousarray(
        qm.transpose(2, 0, 1, 3)).astype(BF16)     # [P, NSGS, 4, GPS*16]

    # window-relative cell values for on-chip one-hot generation
    ch_lo_s, nwin_s = plan["ch_lo_s"], plan["nwin_s"]
    KMAX, cover = plan["KMAX"], plan["cover"]
    NWIN = int(nwin_s.sum())
    cell = c["cell"]
    cadj = np.full((P, NWIN), -1, np.int64)
    wi = 0
    for ct in range(CT):
        for wv in range(int(nwin_s[ct])):
            ch = int(ch_lo_s[ct]) + wv
            if ch < NCHUNKS:
                vals = cell[ch * P:(ch + 1) * P] - ct * WCT
                vals = np.where((vals >= 0) & (vals < WCT), vals, -1)
                cadj[:, wi] = vals
            wi += 1
    out["celladjS"] = cadj.astype(BF16)

    NGRP, cover = plan["NGRP"], plan["cover"]
    KGMAX = plan["KGMAX"]
    qmt = np.zeros((NGRP, KGMAX, P, P), np.float32)
    for g in range(NGRP):
        off = 0
        for t in range(4):
            ch = g * 4 + t
            vals = cell[ch * P:(ch + 1) * P]
            for cw in cover[ch]:
                rv = vals - cw * WCW
                ok = (rv >= 0) & (rv < WCW)
                # matchT layout: [cell-rel partition, node]
                qmt[g, off, rv[ok], np.nonzero(ok)[0]] = 1.0
                off += 1
    out["qmats"] = qmt.astype(BF16)
    return out


def _build(plan, Mv):
    import concourse.bacc as bacc
    import concourse.mybir as mybir
    from concourse.tile import TileContext

    NN, NGRP = plan["NN"], plan["NGRP"]
    NCHUNKS, NSGS = plan["NCHUNKS"], plan["NSGS"]
    CT, W_OUT, CTT = plan["CT"], plan["W_OUT"], plan["CTT"]
    ch_lo_s, nwin_s, cover = plan["ch_lo_s"], plan["nwin_s"], plan["cover"]
    KMAX = plan["KMAX"]
    KGMAX = plan["KGMAX"]
    NWIN = int(nwin_s.sum())
    NWMAX = int(nwin_s.max())
    NIDX = SG * BUNDLE
    NCOL = (SG // P) * BUNDLE        # 32 gather columns per supergroup
    TT = SG // P                     # 8 node-subtiles per supergroup

    nc = bacc.Bacc("TRN2", target_bir_lowering=False, debug=False,
                   num_swdge_queues=4)
    dt = mybir.dt
    AL = mybir.AluOpType
    vtab4 = nc.declare_dram_parameter("vtab4", [Mv // 4, BUNDLE * CV], dt.bfloat16, isOutput=False)
    rtabsT_d = nc.declare_dram_parameter("rtabsT", [CR, W_OUT], dt.bfloat16, isOutput=False)
    vidx_d = nc.declare_dram_parameter("vidx", [P, NSGS, NIDX // 16], dt.int16, isOutput=False)
    qmaskf_d = nc.declare_dram_parameter("qmaskf", [P, NSGS, 4, NCOL], dt.bfloat16, isOutput=False)
    celladjS_d = nc.declare_dram_parameter("celladjS", [P, NWIN], dt.bfloat16, isOutput=False)
    qmats_d = nc.declare_dram_parameter("qmats", [NN // GRP, KGMAX, P, P], dt.bfloat16, isOutput=False)
    a16_d = nc.declare_dram_parameter("a16", [CR, E], dt.bfloat16, isOutput=False)
    wov_d = nc.declare_dram_parameter("wovT", [E, CO], dt.bfloat16, isOutput=False)
    out_d = nc.declare_dram_parameter("out", [CO, W_OUT], dt.float32, isOutput=True)

    with TileContext(nc) as tc:
        with (
            tc.tile_pool(name="res", bufs=1) as res,
            tc.tile_pool(name="x4p", bufs=4) as x4p,
            tc.tile_pool(name="mkp", bufs=2) as mkp,
            tc.tile_pool(name="xp", bufs=3) as xp,
            tc.tile_pool(name="small", bufs=2) as small,
            tc.tile_pool(name="scmp", bufs=3) as scmp,
            tc.tile_pool(name="prodp", bufs=3) as prodp,
            tc.tile_pool(name="xbp", bufs=1) as xbp,
            tc.tile_pool(name="scp", bufs=3) as scp,
            tc.tile_pool(name="psB", bufs=3, space="PSUM") as psB,
            tc.tile_pool(name="psC", bufs=2, space="PSUM") as psC,
            tc.tile_pool(name="psD", bufs=2, space="PSUM") as psD,
        ):
            # ---------- resident loads / constants ----------
            vixall = res.tile([P, NSGS, NIDX // 16], dt.int16)
            nc.sync.dma_start(out=vixall[:], in_=vidx_d[:])
            qmaskf = res.tile([P, NSGS, 4, NCOL], dt.bfloat16)
            nc.sync.dma_start(out=qmaskf[:, 0, :, :], in_=qmaskf_d[:, 0])
            nc.sync.dma_start(out=qmaskf[:, 1:, :, :], in_=qmaskf_d[:, 1:])
            celladjS = res.tile([P, NWIN], dt.bfloat16)
            nc.sync.dma_start(out=celladjS[:], in_=celladjS_d[:])
            a16 = res.tile([CR, E], dt.bfloat16)
            nc.sync.dma_start(out=a16[:], in_=a16_d[:])
            wovT = res.tile([E, CO], dt.bfloat16)
            nc.sync.dma_start(out=wovT[:], in_=wov_d[:])
            iotaRow16 = res.tile([P, WCT], dt.int16)
            nc.gpsimd.iota(iotaRow16[:], pattern=[[1, WCT]], base=0,
                           channel_multiplier=0)
            iotaRow = res.tile([P, WCT], dt.bfloat16)
            nc.scalar.copy(out=iotaRow[:], in_=iotaRow16[:])
            qtable = res.tile([P, CTT, E], dt.bfloat16)
            xbar_g = [xbp.tile([P, TT, E], dt.bfloat16, tag=f"xb{g}",
                                 name=f"xbar{g}")
                      for g in range(NSGS)]

            # ---------- q'-table: qtable[cell,:] = rtabsT[:,cell]^T @ A16 ----
            # built lazily, interleaved with the supergroup loop so the
            # bulk build never sits ahead of sg0 on the PE stream
            qtable_built = [0]

            def build_qtable_to(cw_end):
                for cw in range(qtable_built[0], min(cw_end, CTT)):
                    rsl = small.tile([CR, P], dt.bfloat16, tag="rsl")
                    nc.sync.dma_start(out=rsl[:],
                                      in_=rtabsT_d[:, cw * P:(cw + 1) * P])
                    qp = psB.tile([P, 4, E], dt.float32, tag="psB")
                    nc.tensor.matmul(out=qp[:, 0, :], lhsT=rsl[:],
                                     rhs=a16[:], start=True, stop=True)
                    nc.scalar.copy(out=qtable[:, cw, :], in_=qp[:, 0, :])
                qtable_built[0] = max(qtable_built[0], min(cw_end, CTT))

            # ---------- scatter: one-hot gen on chip + PE accumulate ------
            win_start = np.concatenate(([0], np.cumsum(nwin_s)))

            scmm_ready = {}

            def gen_scmm(ct):
                nw = int(nwin_s[ct])
                wi = int(win_start[ct])
                scmm = scmp.tile([P, NWMAX, WCT], dt.bfloat16, tag="scmm",
                                  name=f"scmm{ct}")
                cax = scmp.tile([P, NWMAX, WCT], dt.bfloat16, tag="cax",
                                name=f"cax{ct}")
                nc.scalar.copy(
                    out=cax[:, 0:nw, :],
                    in_=celladjS[:, wi:wi + nw, None].to_broadcast(
                        [P, nw, WCT]))
                nc.vector.tensor_tensor(
                    out=scmm[:, 0:nw, :],
                    in0=cax[:, 0:nw, :],
                    in1=iotaRow[:, None, :].to_broadcast([P, nw, WCT]),
                    op=AL.is_equal)
                scmm_ready[ct] = scmm

            def emit_scatter(ct):
                nw = int(nwin_s[ct])
                if ct not in scmm_ready:
                    gen_scmm(ct)
                scmm = scmm_ready.pop(ct)
                t1 = psC.tile([CO, WCT], dt.float32, tag="psC", name=f"t1_{ct}")
                for wv in range(nw):
                    ch = min(int(ch_lo_s[ct]) + wv, NCHUNKS - 1)
                    nc.tensor.matmul(out=t1[:],
                                     lhsT=xbar_g[ch // TT][:, ch % TT, :],
                                     rhs=scmm[:, wv, :], start=(wv == 0),
                                     stop=(wv == nw - 1))
                t1s = small.tile([CO, WCT], dt.bfloat16, tag="t1s",
                                 name=f"t1s{ct}")
                nc.scalar.copy(out=t1s[:], in_=t1[:])
                ot = psD.tile([CO, WCT], dt.float32, tag="psD", name=f"ot{ct}")
                nc.tensor.matmul(out=ot[:], lhsT=wovT[:], rhs=t1s[:],
                                 start=True, stop=True)
                osb = small.tile([CO, WCT], dt.float32, tag="osb",
                                 name=f"osb{ct}")
                nc.scalar.copy(out=osb[:], in_=ot[:])
                nc.scalar.dma_start(out=out_d[:, ct * WCT:(ct + 1) * WCT],
                                    in_=osb[:])

            # scatter window ct is ready once every chunk it reads is computed
            ct_ready = []
            for ct in range(CT):
                mx = min(int(ch_lo_s[ct]) + int(nwin_s[ct]) - 1, NCHUNKS - 1)
                ct_ready.append(mx + 1)

            issued = {}

            def issue_gather(sg):
                x4 = x4p.tile([P, NCOL, BUNDLE * CV], dt.bfloat16,
                              tag="x4", name=f"x4_{sg}")
                nh = NIDX // 2
                for h in (0, 1):
                    nc.gpsimd.dma_gather(
                        out_ap=x4[:, h * (NCOL // 2):(h + 1) * (NCOL // 2), :],
                        in_ap=vtab4[:],
                        idxs_ap=vixall[:, sg, h * (nh // 16):(h + 1) * (nh // 16)],
                        num_idxs=nh, num_idxs_reg=nh, elem_size=BUNDLE * CV,
                        single_packet=False, queue_num=(2 * sg + h) % 4)
                issued[sg] = x4

            for sg in range(min(4, NSGS)):
                issue_gather(sg)
            mask_tiles = {}
            m0 = mkp.tile([P, NCOL, BUNDLE * CV], dt.bfloat16,
                          tag="mask4", name="mask4_0")
            nc.scalar.copy(
                out=m0[:].rearrange("p k (q c) -> p q k c", q=4),
                in_=qmaskf[:, 0, :, :, None].to_broadcast(
                    [P, 4, NCOL, CV]))
            mask_tiles[0] = m0
            done_ct = 0
            for sg in range(NSGS):
                if sg not in issued:
                    issue_gather(sg)
                x4 = issued.pop(sg)
                need_cw = max(max(cover[ch]) for ch in
                              range(sg * TT, (sg + 1) * TT)) + 1
                build_qtable_to(need_cw + 8)

                # expand NEXT sg's quad mask first so the scalar engine has
                # it ready before that sg's select needs it
                if sg + 1 < NSGS:
                    nmask = mkp.tile([P, NCOL, BUNDLE * CV], dt.bfloat16,
                                     tag="mask4", name=f"mask4_{sg + 1}")
                    nc.scalar.copy(
                        out=nmask[:].rearrange("p k (q c) -> p q k c", q=4),
                        in_=qmaskf[:, sg + 1, :, :, None].to_broadcast(
                            [P, 4, NCOL, CV]))
                    mask_tiles[sg + 1] = nmask
                mask4 = mask_tiles.pop(sg)
                # pre-generate scatter one-hots for windows completing soon
                nd = done_ct
                cd = (sg + 1) * TT
                while nd < CT and ct_ready[nd] <= cd:
                    gen_scmm(nd)
                    nd += 1
                xsel = mask4
                nc.vector.tensor_tensor(out=xsel[:], in0=x4[:], in1=mask4[:],
                                        op=AL.mult)
                xsg = xp.tile([P, NCOL, CV], dt.bfloat16, tag="x",
                              name=f"xsg{sg}")
                nc.vector.tensor_tensor(out=xsg[:], in0=xsel[:, :, 0:CV],
                                        in1=xsel[:, :, CV:2 * CV], op=AL.add)
                nc.vector.tensor_tensor(out=xsg[:], in0=xsg[:],
                                        in1=xsel[:, :, 2 * CV:3 * CV],
                                        op=AL.add)
                nc.vector.tensor_tensor(out=xsg[:], in0=xsg[:],
                                        in1=xsel[:, :, 3 * CV:4 * CV],
                                        op=AL.add)

                ex = scp.tile([P, NCOL], dt.float32, tag="ex")
                for gs in range(2):
                    g = sg * 2 + gs
                    xv = xsg[:, gs * 16:(gs + 1) * 16, :]

                    qps4 = psB.tile([P, 4, E], dt.float32, tag="psB")
                    kg = sum(len(cover[g * 4 + t]) for t in range(4))
                    mtg = small.tile([P, KGMAX, P], dt.bfloat16, tag="mtg")
                    nc.sync.dma_start(
                        out=mtg[:, 0:kg, :],
                        in_=qmats_d[g, 0:kg].rearrange("k p n -> p k n"))
                    off = 0
                    for t in range(4):
                        ch = g * 4 + t
                        cvr = cover[ch]
                        for ci, cw in enumerate(cvr):
                            nc.tensor.matmul(out=qps4[:, t, :],
                                             lhsT=mtg[:, off + ci, :],
                                             rhs=qtable[:, cw, :],
                                             start=(ci == 0),
                                             stop=(ci == len(cvr) - 1))
                        off += len(cvr)
                    qps4b = scp.tile([P, 4, E], dt.bfloat16, tag="qps4b")
                    nc.scalar.copy(out=qps4b[:], in_=qps4[:])
                    # scores: per-point dot(x, q') over channels
                    prod = prodp.tile([P, 16, CV], dt.bfloat16, tag="prod")
                    nc.vector.tensor_tensor(
                        out=prod[:].rearrange("p (t j) c -> p t j c", t=4),
                        in0=xv.rearrange("p (t j) c -> p t j c", t=4),
                        in1=qps4b[:, :, None, :].to_broadcast([P, 4, 4, E]),
                        op=AL.mult)
                    sc = scp.tile([P, 16], dt.bfloat16, tag="sc")
                    with nc.allow_low_precision(reason="bf16 score reduce"):
                        nc.vector.tensor_reduce(out=sc[:], in_=prod[:],
                                                axis=mybir.AxisListType.X,
                                                op=AL.add)
                    nc.scalar.activation(out=ex[:, gs * 16:(gs + 1) * 16],
                                         in_=sc[:],
                                         func=mybir.ActivationFunctionType.Exp)

                # merged softmax tail + weighted sum at supergroup level
                den = scp.tile([P, TT], dt.float32, tag="den")
                nc.vector.tensor_reduce(
                    out=den[:],
                    in_=ex[:].rearrange("p (t j) -> p t j", t=TT),
                    axis=mybir.AxisListType.X, op=AL.add)
                rec = scp.tile([P, TT], dt.float32, tag="rec")
                nc.vector.reciprocal(out=rec[:], in_=den[:])
                attn = scp.tile([P, NCOL], dt.bfloat16, tag="attn")
                nc.vector.tensor_tensor(
                    out=attn[:].rearrange("p (t j) -> p t j", t=TT),
                    in0=ex[:].rearrange("p (t j) -> p t j", t=TT),
                    in1=rec[:, :, None].to_broadcast([P, TT, 4]),
                    op=AL.mult)
                prod2 = scp.tile([P, NCOL, CV], dt.bfloat16, tag="prod2")
                nc.vector.tensor_tensor(
                    out=prod2[:], in0=xsg[:],
                    in1=attn[:, :, None].to_broadcast([P, NCOL, CV]),
                    op=AL.mult)
                pj = prod2[:].rearrange("p (g t j) c -> p j g t c", g=2, t=4)
                xb = xbar_g[sg][:].rearrange("p (g t) c -> p g t c", g=2)
                nc.vector.tensor_tensor(out=xb, in0=pj[:, 0], in1=pj[:, 1],
                                        op=AL.add)
                nc.vector.tensor_tensor(out=xb, in0=xb, in1=pj[:, 2],
                                        op=AL.add)
                nc.vector.tensor_tensor(out=xb, in0=xb, in1=pj[:, 3],
                                        op=AL.add)

                chunks_done = (sg + 1) * TT
                while done_ct < CT and ct_ready[done_ct] <= chunks_done:
                    emit_scatter(done_ct)
                    done_ct += 1

            build_qtable_to(CTT)
            for ct in range(done_ct, CT):
                emit_scatter(ct)
    nc.compile()
    return nc


def _install_ntff_shim():
    try:
        import antenv.axon_hooks  # noqa
        return
    except ImportError:
        pass
    try:
        from trn_agent_boot.trn_boot import _ntff_profile_via_ctypes
        hook = _ntff_profile_via_ctypes('/opt/axon/libaxon_pjrt.so')
        mod = types.ModuleType("antenv.axon_hooks")
        mod.get_axon_ntff_profile_hook = lambda: hook
        mod.set_axon_ntff_profile_hook = lambda h: None
        import antenv
        antenv.axon_hooks = mod
        sys.modules["antenv.axon_hooks"] = mod
    except Exception:
        pass


def kernel(**inputs):
    v_feat = np.asarray(inputs["v_feat"], np.float32)
    r_feat = np.asarray(inputs["r_feat"], np.float32)
    Wq = np.asarray(inputs["Wq"], np.float32)
    Wk = np.asarray(inputs["Wk"], np.float32)
    Wv = np.asarray(inputs["Wv"], np.float32)
    Wo = np.asarray(inputs["Wo"], np.float32)
    v2p = np.asarray(inputs["v2p_ind"])
    r2p = np.asarray(inputs["r2p_ind"])
    Mv = v_feat.shape[2]
    Mr = r_feat.shape[2]

    plan = _plan(v2p, r2p)
    nc = _build(plan, Mv)

    A16 = (Wq.T @ Wk / np.sqrt(np.float32(E))).astype(BF16)
    WovT16 = np.ascontiguousarray((Wo @ Wv).T).astype(BF16)

    in_maps = []
    vtab_cache = {}
    for c in plan["cores"]:
        arr = _core_arrays(c, plan, v_feat, r_feat, vtab_cache)
        arr["a16"] = A16
        arr["wovT"] = WovT16
        in_maps.append(arr)

    from concourse.bass_utils import run_bass_kernel_spmd
    _install_ntff_shim()
    trace = bool(inputs.get("_trace", False))
    res = run_bass_kernel_spmd(nc, in_maps, core_ids=list(range(8)),
                               trace=trace)
    out = np.zeros((B, CO, Mr), np.float32)
    for ci, c in enumerate(plan["cores"]):
        o = res.results[ci]["out"]
        w = min(c["width"], plan["W_OUT"])
        out[c["b"], :, c["clo"]:c["clo"] + w] = o[:, :w]
    kernel.last_exec_time_ns = res.exec_time_ns
    return out


kernel.last_exec_time_ns = None


# revision 51
# speedup vs baseline: 1.0970x; 1.0665x over previous
"""Trainium2 Bass kernel for nn_AttentionBlock2 (gnn_message_passing).

8 NeuronCores, SPMD, no collectives:
  - 2 batches x 4 cores; within a batch, nodes sorted by r-cell and split
    into 4 contiguous cell ranges (disjoint output slices per core).
  - v-gather: dma_gather (SWDGE, 4 queues) of quad-packed bf16 rows
    (4 feature rows per 512B table row -> int16 indexable); on-chip 4-way
    select as one all-bf16 multiply + 3 adds (DVE 2x mode) against a
    scalar-engine-expanded quad mask.
  - Math refactor: q' = (Wq^T Wk / sqrt(E)) r ; output proj Wov = Wo@Wv
    applied after the scatter.
  - One-hot match matrices generated on chip (iota + is_equal) instead of
    streamed from HBM; q'-expansion matmuls accumulate straight into PSUM
    (t-major column layout, no per-t replication copies).
  - scatter-add: PE matmuls of xbar vs one-hot match tiles, PSUM-
    accumulated per 256-cell output window, interleaved with compute.
"""

import sys
import types
import numpy as np
import ml_dtypes

B = 2
CV = 64
CR = 20
E = 64
CO = 64
BUNDLE = 4
P = 128
PER_B = 4
SG = 1024            # nodes per dma_gather call (SG*4 = 4096 idx)
GRP = 512            # nodes per compute group
WCT = 256            # scatter window width (cells)
WCW = 128            # q'-expansion window width (cells)
NEG = -(10 ** 9)

BF16 = ml_dtypes.bfloat16


def _plan(v2p, r2p):
    """Data-dependent but core-uniform plan."""
    Nn = r2p.shape[1]
    cores = []
    for b in range(B):
        cells = r2p[b, :, 0].astype(np.int64)
        order = np.argsort(cells, kind="stable")
        sc = cells[order]
        bounds = []
        for k in range(1, PER_B):
            c = sc[k * Nn // PER_B]
            bounds.append(int(np.searchsorted(sc, c)))
        pb = [0] + bounds + [Nn]
        for pi in range(PER_B):
            lo, hi = pb[pi], pb[pi + 1]
            nodes = order[lo:hi]
            clo = int(sc[lo])
            cores.append(dict(b=b, nodes=nodes, clo=clo,
                              width=int(sc[hi - 1]) + 1 - clo))
    nmax = max(len(c["nodes"]) for c in cores)
    NGRP = -(-nmax // GRP)
    gps = SG // GRP
    if NGRP % gps:
        NGRP += gps - NGRP % gps
    NN = NGRP * GRP
    NCHUNKS = NN // P
    NSGS = NN // SG
    wmax = max(c["width"] for c in cores)
    CT = -(-wmax // WCT)
    W_OUT = CT * WCT
    CTT = W_OUT // WCW

    for c in cores:
        n = len(c["nodes"])
        c["n"] = n
        cr = np.full(NN, NEG, np.int64)
        cr[:n] = r2p[c["b"], c["nodes"], 0].astype(np.int64) - c["clo"]
        c["cell"] = cr
        vr = np.zeros((NN, BUNDLE), np.int64)
        vr[:n] = v2p[c["b"], :, 0].reshape(Nn, BUNDLE)[c["nodes"]]
        c["vrow"] = vr

    ch_lo_s = np.full(CT, 10 ** 9, np.int64)
    ch_hi_s = np.zeros(CT, np.int64)
    ch_lo_t = np.full(CTT, 10 ** 9, np.int64)
    ch_hi_t = np.zeros(CTT, np.int64)
    for c in cores:
        cr = c["cell"]
        valid = cr > NEG
        for W, lo_arr, hi_arr, CN in ((WCT, ch_lo_s, ch_hi_s, CT),
                                      (WCW, ch_lo_t, ch_hi_t, CTT)):
            w_of = np.where(valid, cr // W, -1)
            for wi in range(CN):
                idx = np.nonzero(w_of == wi)[0]
                if len(idx):
                    lo_arr[wi] = min(lo_arr[wi], idx[0] // P)
                    hi_arr[wi] = max(hi_arr[wi], idx[-1] // P + 1)
    ch_lo_s = np.where(ch_lo_s > ch_hi_s, 0, ch_lo_s)
    nwin_s = np.maximum(ch_hi_s - ch_lo_s, 1).astype(np.int64)
    ch_lo_t = np.where(ch_lo_t > ch_hi_t, 0, ch_lo_t)
    nwin_t = np.maximum(ch_hi_t - ch_lo_t, 1).astype(np.int64)

    cover = [[] for _ in range(NCHUNKS)]
    for cw in range(CTT):
        if ch_hi_t[cw] == 0:      # no core has nodes in this window
            continue
        for ch in range(int(ch_lo_t[cw]), int(ch_lo_t[cw] + nwin_t[cw])):
            if 0 <= ch < NCHUNKS:
                cover[ch].append(cw)
    for ch in range(NCHUNKS):
        if not cover[ch]:
            cover[ch].append(0)
        lo, hi = min(cover[ch]), max(cover[ch])
        cover[ch] = list(range(lo, hi + 1))

    KMAX = max(len(cv) for cv in cover)
    ngrp2 = NN // GRP
    KGMAX = max(sum(len(cover[g * 4 + t]) for t in range(4))
                for g in range(ngrp2))
    return dict(cores=cores, NN=NN, NGRP=NGRP, NCHUNKS=NCHUNKS, NSGS=NSGS,
                KGMAX=KGMAX,
                CT=CT, W_OUT=W_OUT, CTT=CTT,
                ch_lo_s=ch_lo_s, nwin_s=nwin_s, cover=cover, KMAX=KMAX)


def _core_arrays(c, plan, v_feat, r_feat, vtab_cache):
    NN, NSGS, CT, W_OUT = plan["NN"], plan["NSGS"], plan["CT"], plan["W_OUT"]
    NCHUNKS = plan["NCHUNKS"]
    b = c["b"]
    out = {}
    if b not in vtab_cache:
        vt = np.ascontiguousarray(v_feat[b].T).astype(BF16)   # [Mv, 64]
        vtab_cache[b] = np.ascontiguousarray(vt.reshape(-1, BUNDLE * CV))
    out["vtab4"] = vtab_cache[b]
    rtT = np.zeros((CR, W_OUT), np.float32)
    w = min(c["width"], W_OUT)
    rtT[:, :w] = r_feat[b][:, c["clo"]: c["clo"] + w]
    out["rtabsT"] = np.ascontiguousarray(rtT).astype(BF16)

    vr = c["vrow"]
    NIDX = SG * BUNDLE
    # position (sg, k*P + p): k = gs*16 + t*4 + j (t-major), node =
    # sg*SG + gs*GRP + t*P + p, bundle member j.
    nodes_all = np.arange(NN)
    gps = SG // GRP
    gs_of = (nodes_all // GRP) % gps
    t_of = (nodes_all % GRP) // P
    p_of = nodes_all % P
    sg_of = nodes_all // SG
    rows = vr                                   # [NN, 4]
    k_of = gs_of * 16 + t_of * 4                # [NN]
    vidx = np.zeros((NSGS, NIDX), np.int64)
    quad = np.zeros((NSGS, P, (SG // P) * 4), np.uint8)
    for j in range(BUNDLE):
        kj = k_of + j
        vidx[sg_of, kj * P + p_of] = rows[:, j] // 4
        quad[sg_of, p_of, kj] = rows[:, j] % 4
    assert vidx.max() < 32768, "v row index exceeds int16 quad range"
    viw = np.tile(vidx.reshape(NSGS, NIDX // 16, 16).transpose(0, 2, 1),
                  (1, 8, 1))                       # [NSGS, P, NIDX//16]
    out["vidx"] = np.ascontiguousarray(
        viw.transpose(1, 0, 2)).astype(np.int16)   # [P, NSGS, NIDX//16]

    qm = np.zeros((NSGS, 4, P, (SG // P) * 4), np.float32)
    for qi in (0, 1, 2, 3):
        qm[:, qi] = (quad == qi)
    out["qmaskf"] = np.ascontiguousarray(
        qm.transpose(2, 0, 1, 3)).astype(BF16)     # [P, NSGS, 4, GPS*16]

    # window-relative cell values for on-chip one-hot generation
    ch_lo_s, nwin_s = plan["ch_lo_s"], plan["nwin_s"]
    KMAX, cover = plan["KMAX"], plan["cover"]
    NWIN = int(nwin_s.sum())
    cell = c["cell"]
    cadj = np.full((P, NWIN), -1, np.int64)
    wi = 0
    for ct in range(CT):
        for wv in range(int(nwin_s[ct])):
            ch = int(ch_lo_s[ct]) + wv
            if ch < NCHUNKS:
                vals = cell[ch * P:(ch + 1) * P] - ct * WCT
                vals = np.where((vals >= 0) & (vals < WCT), vals, -1)
                cadj[:, wi] = vals
            wi += 1
    out["celladjS"] = cadj.astype(BF16)

    NGRP, cover = plan["NGRP"], plan["cover"]
    KGMAX = plan["KGMAX"]
    qmt = np.zeros((NGRP, KGMAX, P, P), np.float32)
    for g in range(NGRP):
        off = 0
        for t in range(4):
            ch = g * 4 + t
            vals = cell[ch * P:(ch + 1) * P]
            for cw in cover[ch]:
                rv = vals - cw * WCW
                ok = (rv >= 0) & (rv < WCW)
                # matchT layout: [cell-rel partition, node]
                qmt[g, off, rv[ok], np.nonzero(ok)[0]] = 1.0
                off += 1
    out["qmats"] = qmt.astype(BF16)
    return out


def _build(plan, Mv):
    import concourse.bacc as bacc
    import concourse.mybir as mybir
    from concourse.tile import TileContext

    NN, NGRP = plan["NN"], plan["NGRP"]
    NCHUNKS, NSGS = plan["NCHUNKS"], plan["NSGS"]
    CT, W_OUT, CTT = plan["CT"], plan["W_OUT"], plan["CTT"]
    ch_lo_s, nwin_s, cover = plan["ch_lo_s"], plan["nwin_s"], plan["cover"]
    KMAX = plan["KMAX"]
    KGMAX = plan["KGMAX"]
    NWIN = int(nwin_s.sum())
    NWMAX = int(nwin_s.max())
    NIDX = SG * BUNDLE
    NCOL = (SG // P) * BUNDLE        # 32 gather columns per supergroup
    TT = SG // P                     # 8 node-subtiles per supergroup

    nc = bacc.Bacc("TRN2", target_bir_lowering=False, debug=False,
                   num_swdge_queues=4)
    dt = mybir.dt
    AL = mybir.AluOpType
    vtab4 = nc.declare_dram_parameter("vtab4", [Mv // 4, BUNDLE * CV], dt.bfloat16, isOutput=False)
    rtabsT_d = nc.declare_dram_parameter("rtabsT", [CR, W_OUT], dt.bfloat16, isOutput=False)
    vidx_d = nc.declare_dram_parameter("vidx", [P, NSGS, NIDX // 16], dt.int16, isOutput=False)
    qmaskf_d = nc.declare_dram_parameter("qmaskf", [P, NSGS, 4, NCOL], dt.bfloat16, isOutput=False)
    celladjS_d = nc.declare_dram_parameter("celladjS", [P, NWIN], dt.bfloat16, isOutput=False)
    qmats_d = nc.declare_dram_parameter("qmats", [NN // GRP, KGMAX, P, P], dt.bfloat16, isOutput=False)
    a16_d = nc.declare_dram_parameter("a16", [CR, E], dt.bfloat16, isOutput=False)
    wov_d = nc.declare_dram_parameter("wovT", [E, CO], dt.bfloat16, isOutput=False)
    out_d = nc.declare_dram_parameter("out", [CO, W_OUT], dt.float32, isOutput=True)

    with TileContext(nc) as tc:
        with (
            tc.tile_pool(name="res", bufs=1) as res,
            tc.tile_pool(name="x4p", bufs=4) as x4p,
            tc.tile_pool(name="mkp", bufs=2) as mkp,
            tc.tile_pool(name="xp", bufs=3) as xp,
            tc.tile_pool(name="small", bufs=2) as small,
            tc.tile_pool(name="scmp", bufs=3) as scmp,
            tc.tile_pool(name="prodp", bufs=3) as prodp,
            tc.tile_pool(name="xbp", bufs=1) as xbp,
            tc.tile_pool(name="scp", bufs=3) as scp,
            tc.tile_pool(name="psB", bufs=3, space="PSUM") as psB,
            tc.tile_pool(name="psC", bufs=2, space="PSUM") as psC,
            tc.tile_pool(name="psD", bufs=2, space="PSUM") as psD,
        ):
            # ---------- resident loads / constants ----------
            vixall = res.tile([P, NSGS, NIDX // 16], dt.int16)
            nc.sync.dma_start(out=vixall[:], in_=vidx_d[:])
            qmaskf = res.tile([P, NSGS, 4, NCOL], dt.bfloat16)
            nc.sync.dma_start(out=qmaskf[:, 0, :, :], in_=qmaskf_d[:, 0])
            nc.sync.dma_start(out=qmaskf[:, 1:, :, :], in_=qmaskf_d[:, 1:])
            celladjS = res.tile([P, NWIN], dt.bfloat16)
            nc.sync.dma_start(out=celladjS[:], in_=celladjS_d[:])
            a16 = res.tile([CR, E], dt.bfloat16)
            nc.sync.dma_start(out=a16[:], in_=a16_d[:])
            wovT = res.tile([E, CO], dt.bfloat16)
            nc.sync.dma_start(out=wovT[:], in_=wov_d[:])
            iotaRow16 = res.tile([P, WCT], dt.int16)
            nc.gpsimd.iota(iotaRow16[:], pattern=[[1, WCT]], base=0,
                           channel_multiplier=0)
            iotaRow = res.tile([P, WCT], dt.bfloat16)
            nc.scalar.copy(out=iotaRow[:], in_=iotaRow16[:])
            qtable = res.tile([P, CTT, E], dt.bfloat16)
            xbar_g = [xbp.tile([P, TT, E], dt.bfloat16, tag=f"xb{g}",
                                 name=f"xbar{g}")
                      for g in range(NSGS)]

            # ---------- q'-table: qtable[cell,:] = rtabsT[:,cell]^T @ A16 ----
            # built lazily, interleaved with the supergroup loop so the
            # bulk build never sits ahead of sg0 on the PE stream
            qtable_built = [0]

            def build_qtable_to(cw_end):
                for cw in range(qtable_built[0], min(cw_end, CTT)):
                    rsl = small.tile([CR, P], dt.bfloat16, tag="rsl")
                    nc.sync.dma_start(out=rsl[:],
                                      in_=rtabsT_d[:, cw * P:(cw + 1) * P])
                    qp = psB.tile([P, 4, E], dt.float32, tag="psB")
                    nc.tensor.matmul(out=qp[:, 0, :], lhsT=rsl[:],
                                     rhs=a16[:], start=True, stop=True)
                    nc.scalar.copy(out=qtable[:, cw, :], in_=qp[:, 0, :])
                qtable_built[0] = max(qtable_built[0], min(cw_end, CTT))

            # ---------- scatter: one-hot gen on chip + PE accumulate ------
            win_start = np.concatenate(([0], np.cumsum(nwin_s)))

            scmm_ready = {}

            def gen_scmm(ct):
                nw = int(nwin_s[ct])
                wi = int(win_start[ct])
                scmm = scmp.tile([P, NWMAX, WCT], dt.bfloat16, tag="scmm",
                                  name=f"scmm{ct}")
                cax = scmp.tile([P, NWMAX, WCT], dt.bfloat16, tag="cax",
                                name=f"cax{ct}")
                nc.scalar.copy(
                    out=cax[:, 0:nw, :],
                    in_=celladjS[:, wi:wi + nw, None].to_broadcast(
                        [P, nw, WCT]))
                nc.vector.tensor_tensor(
                    out=scmm[:, 0:nw, :],
                    in0=cax[:, 0:nw, :],
                    in1=iotaRow[:, None, :].to_broadcast([P, nw, WCT]),
                    op=AL.is_equal)
                scmm_ready[ct] = scmm

            def emit_scatter(ct):
                nw = int(nwin_s[ct])
                if ct not in scmm_ready:
                    gen_scmm(ct)
                scmm = scmm_ready.pop(ct)
                t1 = psC.tile([CO, WCT], dt.float32, tag="psC", name=f"t1_{ct}")
                for wv in range(nw):
                    ch = min(int(ch_lo_s[ct]) + wv, NCHUNKS - 1)
                    nc.tensor.matmul(out=t1[:],
                                     lhsT=xbar_g[ch // TT][:, ch % TT, :],
                                     rhs=scmm[:, wv, :], start=(wv == 0),
                                     stop=(wv == nw - 1))
                t1s = small.tile([CO, WCT], dt.bfloat16, tag="t1s",
                                 name=f"t1s{ct}")
                nc.scalar.copy(out=t1s[:], in_=t1[:])
                ot = psD.tile([CO, WCT], dt.float32, tag="psD", name=f"ot{ct}")
                nc.tensor.matmul(out=ot[:], lhsT=wovT[:], rhs=t1s[:],
                                 start=True, stop=True)
                osb = small.tile([CO, WCT], dt.float32, tag="osb",
                                 name=f"osb{ct}")
                nc.scalar.copy(out=osb[:], in_=ot[:])
                nc.scalar.dma_start(out=out_d[:, ct * WCT:(ct + 1) * WCT],
                                    in_=osb[:])

            # scatter window ct is ready once every chunk it reads is computed
            ct_ready = []
            for ct in range(CT):
                mx = min(int(ch_lo_s[ct]) + int(nwin_s[ct]) - 1, NCHUNKS - 1)
                ct_ready.append(mx + 1)

            issued = {}

            def issue_gather(sg):
                x4 = x4p.tile([P, NCOL, BUNDLE * CV], dt.bfloat16,
                              tag="x4", name=f"x4_{sg}")
                nh = NIDX // 2
                for h in (0, 1):
                    nc.gpsimd.dma_gather(
                        out_ap=x4[:, h * (NCOL // 2):(h + 1) * (NCOL // 2), :],
                        in_ap=vtab4[:],
                        idxs_ap=vixall[:, sg, h * (nh // 16):(h + 1) * (nh // 16)],
                        num_idxs=nh, num_idxs_reg=nh, elem_size=BUNDLE * CV,
                        single_packet=False, queue_num=(2 * sg + h) % 4)
                issued[sg] = x4

            for sg in range(min(4, NSGS)):
                issue_gather(sg)
            mask_tiles = {}
            m0 = mkp.tile([P, NCOL, BUNDLE * CV], dt.bfloat16,
                          tag="mask4", name="mask4_0")
            nc.scalar.copy(
                out=m0[:].rearrange("p k (q c) -> p q k c", q=4),
                in_=qmaskf[:, 0, :, :, None].to_broadcast(
                    [P, 4, NCOL, CV]))
            mask_tiles[0] = m0
            done_ct = 0
            for sg in range(NSGS):
                if sg not in issued:
                    issue_gather(sg)
                x4 = issued.pop(sg)
                need_cw = max(max(cover[ch]) for ch in
                              range(sg * TT, (sg + 1) * TT)) + 1
                build_qtable_to(need_cw + 8)

                # expand NEXT sg's quad mask first so the scalar engine has
                # it ready before that sg's select needs it
                if sg + 1 < NSGS:
                    nmask = mkp.tile([P, NCOL, BUNDLE * CV], dt.bfloat16,
                                     tag="mask4", name=f"mask4_{sg + 1}")
                    nc.scalar.copy(
                        out=nmask[:].rearrange("p k (q c) -> p q k c", q=4),
                        in_=qmaskf[:, sg + 1, :, :, None].to_broadcast(
                            [P, 4, NCOL, CV]))
                    mask_tiles[sg + 1] = nmask
                mask4 = mask_tiles.pop(sg)
                # pre-generate scatter one-hots for windows completing soon
                nd = done_ct
                cd = (sg + 1) * TT
                while nd < CT and ct_ready[nd] <= cd:
                    gen_scmm(nd)
                    nd += 1
                xsel = mask4
                xsg = xp.tile([P, NCOL, CV], dt.bfloat16, tag="x",
                              name=f"xsg{sg}")
                HC = NCOL // 2
                for h in (0, 1):
                    xse = xsel[:, h * HC:(h + 1) * HC, :]
                    xsh = xsg[:, h * HC:(h + 1) * HC, :]
                    nc.vector.tensor_tensor(
                        out=xse, in0=x4[:, h * HC:(h + 1) * HC, :],
                        in1=xse, op=AL.mult)
                    nc.vector.tensor_tensor(out=xsh, in0=xse[:, :, 0:CV],
                                            in1=xse[:, :, CV:2 * CV],
                                            op=AL.add)
                    nc.vector.tensor_tensor(out=xsh, in0=xsh,
                                            in1=xse[:, :, 2 * CV:3 * CV],
                                            op=AL.add)
                    nc.vector.tensor_tensor(out=xsh, in0=xsh,
                                            in1=xse[:, :, 3 * CV:4 * CV],
                                            op=AL.add)

                ex = scp.tile([P, NCOL], dt.float32, tag="ex")
                for gs in range(2):
                    g = sg * 2 + gs
                    xv = xsg[:, gs * 16:(gs + 1) * 16, :]

                    qps4 = psB.tile([P, 4, E], dt.float32, tag="psB")
                    kg = sum(len(cover[g * 4 + t]) for t in range(4))
                    mtg = small.tile([P, KGMAX, P], dt.bfloat16, tag="mtg")
                    nc.sync.dma_start(
                        out=mtg[:, 0:kg, :],
                        in_=qmats_d[g, 0:kg].rearrange("k p n -> p k n"))
                    off = 0
                    for t in range(4):
                        ch = g * 4 + t
                        cvr = cover[ch]
                        for ci, cw in enumerate(cvr):
                            nc.tensor.matmul(out=qps4[:, t, :],
                                             lhsT=mtg[:, off + ci, :],
                                             rhs=qtable[:, cw, :],
                                             start=(ci == 0),
                                             stop=(ci == len(cvr) - 1))
                        off += len(cvr)
                    qps4b = scp.tile([P, 4, E], dt.bfloat16, tag="qps4b")
                    nc.scalar.copy(out=qps4b[:], in_=qps4[:])
                    # scores: per-point dot(x, q') over channels
                    prod = prodp.tile([P, 16, CV], dt.bfloat16, tag="prod")
                    nc.vector.tensor_tensor(
                        out=prod[:].rearrange("p (t j) c -> p t j c", t=4),
                        in0=xv.rearrange("p (t j) c -> p t j c", t=4),
                        in1=qps4b[:, :, None, :].to_broadcast([P, 4, 4, E]),
                        op=AL.mult)
                    sc = scp.tile([P, 16], dt.bfloat16, tag="sc")
                    with nc.allow_low_precision(reason="bf16 score reduce"):
                        nc.vector.tensor_reduce(out=sc[:], in_=prod[:],
                                                axis=mybir.AxisListType.X,
                                                op=AL.add)
                    nc.scalar.activation(out=ex[:, gs * 16:(gs + 1) * 16],
                                         in_=sc[:],
                                         func=mybir.ActivationFunctionType.Exp)

                # merged softmax tail + weighted sum at supergroup level
                den = scp.tile([P, TT], dt.float32, tag="den")
                nc.vector.tensor_reduce(
                    out=den[:],
                    in_=ex[:].rearrange("p (t j) -> p t j", t=TT),
                    axis=mybir.AxisListType.X, op=AL.add)
                rec = scp.tile([P, TT], dt.float32, tag="rec")
                nc.vector.reciprocal(out=rec[:], in_=den[:])
                attn = scp.tile([P, NCOL], dt.bfloat16, tag="attn")
                nc.vector.tensor_tensor(
                    out=attn[:].rearrange("p (t j) -> p t j", t=TT),
                    in0=ex[:].rearrange("p (t j) -> p t j", t=TT),
                    in1=rec[:, :, None].to_broadcast([P, TT, 4]),
                    op=AL.mult)
                prod2 = scp.tile([P, NCOL, CV], dt.bfloat16, tag="prod2")
                nc.vector.tensor_tensor(
                    out=prod2[:], in0=xsg[:],
                    in1=attn[:, :, None].to_broadcast([P, NCOL, CV]),
                    op=AL.mult)
                pj = prod2[:].rearrange("p (g t j) c -> p j g t c", g=2, t=4)
                xb = xbar_g[sg][:].rearrange("p (g t) c -> p g t c", g=2)
                nc.vector.tensor_tensor(out=xb, in0=pj[:, 0], in1=pj[:, 1],
                                        op=AL.add)
                nc.vector.tensor_tensor(out=xb, in0=xb, in1=pj[:, 2],
                                        op=AL.add)
                nc.vector.tensor_tensor(out=xb, in0=xb, in1=pj[:, 3],
                                        op=AL.add)

                chunks_done = (sg + 1) * TT
                while done_ct < CT and ct_ready[done_ct] <= chunks_done:
                    emit_scatter(done_ct)
                    done_ct += 1

            build_qtable_to(CTT)
            for ct in range(done_ct, CT):
                emit_scatter(ct)
    nc.compile()
    return nc


def _install_ntff_shim():
    try:
        import antenv.axon_hooks  # noqa
        return
    except ImportError:
        pass
    try:
        from trn_agent_boot.trn_boot import _ntff_profile_via_ctypes
        hook = _ntff_profile_via_ctypes('/opt/axon/libaxon_pjrt.so')
        mod = types.ModuleType("antenv.axon_hooks")
        mod.get_axon_ntff_profile_hook = lambda: hook
        mod.set_axon_ntff_profile_hook = lambda h: None
        import antenv
        antenv.axon_hooks = mod
        sys.modules["antenv.axon_hooks"] = mod
    except Exception:
        pass


def kernel(**inputs):
    v_feat = np.asarray(inputs["v_feat"], np.float32)
    r_feat = np.asarray(inputs["r_feat"], np.float32)
    Wq = np.asarray(inputs["Wq"], np.float32)
    Wk = np.asarray(inputs["Wk"], np.float32)
    Wv = np.asarray(inputs["Wv"], np.float32)
    Wo = np.asarray(inputs["Wo"], np.float32)
    v2p = np.asarray(inputs["v2p_ind"])
    r2p = np.asarray(inputs["r2p_ind"])
    Mv = v_feat.shape[2]
    Mr = r_feat.shape[2]

    plan = _plan(v2p, r2p)
    nc = _build(plan, Mv)

    A16 = (Wq.T @ Wk / np.sqrt(np.float32(E))).astype(BF16)
    WovT16 = np.ascontiguousarray((Wo @ Wv).T).astype(BF16)

    in_maps = []
    vtab_cache = {}
    for c in plan["cores"]:
        arr = _core_arrays(c, plan, v_feat, r_feat, vtab_cache)
        arr["a16"] = A16
        arr["wovT"] = WovT16
        in_maps.append(arr)

    from concourse.bass_utils import run_bass_kernel_spmd
    _install_ntff_shim()
    trace = bool(inputs.get("_trace", False))
    res = run_bass_kernel_spmd(nc, in_maps, core_ids=list(range(8)),
                               trace=trace)
    out = np.zeros((B, CO, Mr), np.float32)
    for ci, c in enumerate(plan["cores"]):
        o = res.results[ci]["out"]
        w = min(c["width"], plan["W_OUT"])
        out[c["b"], :, c["clo"]:c["clo"] + w] = o[:, :w]
    kernel.last_exec_time_ns = res.exec_time_ns
    return out


kernel.last_exec_time_ns = None


# revision 52
# speedup vs baseline: 1.1105x; 1.0123x over previous
"""Trainium2 Bass kernel for nn_AttentionBlock2 (gnn_message_passing).

8 NeuronCores, SPMD, no collectives:
  - 2 batches x 4 cores; within a batch, nodes sorted by r-cell and split
    into 4 contiguous cell ranges (disjoint output slices per core).
  - v-gather: dma_gather (SWDGE, 4 queues) of quad-packed bf16 rows
    (4 feature rows per 512B table row -> int16 indexable); on-chip 4-way
    select as one all-bf16 multiply + 3 adds (DVE 2x mode) against a
    scalar-engine-expanded quad mask.
  - Math refactor: q' = (Wq^T Wk / sqrt(E)) r ; output proj Wov = Wo@Wv
    applied after the scatter.
  - One-hot match matrices generated on chip (iota + is_equal) instead of
    streamed from HBM; q'-expansion matmuls accumulate straight into PSUM
    (t-major column layout, no per-t replication copies).
  - scatter-add: PE matmuls of xbar vs one-hot match tiles, PSUM-
    accumulated per 256-cell output window, interleaved with compute.
"""

import sys
import types
import numpy as np
import ml_dtypes

B = 2
CV = 64
CR = 20
E = 64
CO = 64
BUNDLE = 4
P = 128
PER_B = 4
SG = 1024            # nodes per dma_gather call (SG*4 = 4096 idx)
GRP = 512            # nodes per compute group
WCT = 256            # scatter window width (cells)
WCW = 128            # q'-expansion window width (cells)
NEG = -(10 ** 9)

BF16 = ml_dtypes.bfloat16


def _plan(v2p, r2p):
    """Data-dependent but core-uniform plan."""
    Nn = r2p.shape[1]
    cores = []
    for b in range(B):
        cells = r2p[b, :, 0].astype(np.int64)
        order = np.argsort(cells, kind="stable")
        sc = cells[order]
        bounds = []
        for k in range(1, PER_B):
            c = sc[k * Nn // PER_B]
            bounds.append(int(np.searchsorted(sc, c)))
        pb = [0] + bounds + [Nn]
        for pi in range(PER_B):
            lo, hi = pb[pi], pb[pi + 1]
            nodes = order[lo:hi]
            clo = int(sc[lo])
            cores.append(dict(b=b, nodes=nodes, clo=clo,
                              width=int(sc[hi - 1]) + 1 - clo))
    nmax = max(len(c["nodes"]) for c in cores)
    NGRP = -(-nmax // GRP)
    gps = SG // GRP
    if NGRP % gps:
        NGRP += gps - NGRP % gps
    NN = NGRP * GRP
    NCHUNKS = NN // P
    NSGS = NN // SG
    wmax = max(c["width"] for c in cores)
    CT = -(-wmax // WCT)
    W_OUT = CT * WCT
    CTT = W_OUT // WCW

    for c in cores:
        n = len(c["nodes"])
        c["n"] = n
        cr = np.full(NN, NEG, np.int64)
        cr[:n] = r2p[c["b"], c["nodes"], 0].astype(np.int64) - c["clo"]
        c["cell"] = cr
        vr = np.zeros((NN, BUNDLE), np.int64)
        vr[:n] = v2p[c["b"], :, 0].reshape(Nn, BUNDLE)[c["nodes"]]
        c["vrow"] = vr

    ch_lo_s = np.full(CT, 10 ** 9, np.int64)
    ch_hi_s = np.zeros(CT, np.int64)
    ch_lo_t = np.full(CTT, 10 ** 9, np.int64)
    ch_hi_t = np.zeros(CTT, np.int64)
    for c in cores:
        cr = c["cell"]
        valid = cr > NEG
        for W, lo_arr, hi_arr, CN in ((WCT, ch_lo_s, ch_hi_s, CT),
                                      (WCW, ch_lo_t, ch_hi_t, CTT)):
            w_of = np.where(valid, cr // W, -1)
            for wi in range(CN):
                idx = np.nonzero(w_of == wi)[0]
                if len(idx):
                    lo_arr[wi] = min(lo_arr[wi], idx[0] // P)
                    hi_arr[wi] = max(hi_arr[wi], idx[-1] // P + 1)
    ch_lo_s = np.where(ch_lo_s > ch_hi_s, 0, ch_lo_s)
    nwin_s = np.maximum(ch_hi_s - ch_lo_s, 1).astype(np.int64)
    ch_lo_t = np.where(ch_lo_t > ch_hi_t, 0, ch_lo_t)
    nwin_t = np.maximum(ch_hi_t - ch_lo_t, 1).astype(np.int64)

    cover = [[] for _ in range(NCHUNKS)]
    for cw in range(CTT):
        if ch_hi_t[cw] == 0:      # no core has nodes in this window
            continue
        for ch in range(int(ch_lo_t[cw]), int(ch_lo_t[cw] + nwin_t[cw])):
            if 0 <= ch < NCHUNKS:
                cover[ch].append(cw)
    for ch in range(NCHUNKS):
        if not cover[ch]:
            cover[ch].append(0)
        lo, hi = min(cover[ch]), max(cover[ch])
        cover[ch] = list(range(lo, hi + 1))

    KMAX = max(len(cv) for cv in cover)
    ngrp2 = NN // GRP
    KGMAX = max(sum(len(cover[g * 4 + t]) for t in range(4))
                for g in range(ngrp2))
    return dict(cores=cores, NN=NN, NGRP=NGRP, NCHUNKS=NCHUNKS, NSGS=NSGS,
                KGMAX=KGMAX,
                CT=CT, W_OUT=W_OUT, CTT=CTT,
                ch_lo_s=ch_lo_s, nwin_s=nwin_s, cover=cover, KMAX=KMAX)


def _core_arrays(c, plan, v_feat, r_feat, vtab_cache):
    NN, NSGS, CT, W_OUT = plan["NN"], plan["NSGS"], plan["CT"], plan["W_OUT"]
    NCHUNKS = plan["NCHUNKS"]
    b = c["b"]
    out = {}
    if b not in vtab_cache:
        vt = np.ascontiguousarray(v_feat[b].T).astype(BF16)   # [Mv, 64]
        vtab_cache[b] = np.ascontiguousarray(vt.reshape(-1, BUNDLE * CV))
    out["vtab4"] = vtab_cache[b]
    rtT = np.zeros((CR, W_OUT), np.float32)
    w = min(c["width"], W_OUT)
    rtT[:, :w] = r_feat[b][:, c["clo"]: c["clo"] + w]
    out["rtabsT"] = np.ascontiguousarray(rtT).astype(BF16)

    vr = c["vrow"]
    NIDX = SG * BUNDLE
    # position (sg, k*P + p): k = gs*16 + t*4 + j (t-major), node =
    # sg*SG + gs*GRP + t*P + p, bundle member j.
    nodes_all = np.arange(NN)
    gps = SG // GRP
    gs_of = (nodes_all // GRP) % gps
    t_of = (nodes_all % GRP) // P
    p_of = nodes_all % P
    sg_of = nodes_all // SG
    rows = vr                                   # [NN, 4]
    k_of = gs_of * 16 + t_of * 4                # [NN]
    vidx = np.zeros((NSGS, NIDX), np.int64)
    quad = np.zeros((NSGS, P, (SG // P) * 4), np.uint8)
    for j in range(BUNDLE):
        kj = k_of + j
        vidx[sg_of, kj * P + p_of] = rows[:, j] // 4
        quad[sg_of, p_of, kj] = rows[:, j] % 4
    assert vidx.max() < 32768, "v row index exceeds int16 quad range"
    viw = np.tile(vidx.reshape(NSGS, NIDX // 16, 16).transpose(0, 2, 1),
                  (1, 8, 1))                       # [NSGS, P, NIDX//16]
    out["vidx"] = np.ascontiguousarray(
        viw.transpose(1, 0, 2)).astype(np.int16)   # [P, NSGS, NIDX//16]

    qm = np.zeros((NSGS, 4, P, (SG // P) * 4), np.float32)
    for qi in (0, 1, 2, 3):
        qm[:, qi] = (quad == qi)
    out["qmaskf"] = np.ascontiguousarray(
        qm.transpose(2, 0, 1, 3)).astype(BF16)     # [P, NSGS, 4, GPS*16]

    # window-relative cell values for on-chip one-hot generation
    ch_lo_s, nwin_s = plan["ch_lo_s"], plan["nwin_s"]
    KMAX, cover = plan["KMAX"], plan["cover"]
    NWIN = int(nwin_s.sum())
    cell = c["cell"]
    cadj = np.full((P, NWIN), -1, np.int64)
    wi = 0
    for ct in range(CT):
        for wv in range(int(nwin_s[ct])):
            ch = int(ch_lo_s[ct]) + wv
            if ch < NCHUNKS:
                vals = cell[ch * P:(ch + 1) * P] - ct * WCT
                vals = np.where((vals >= 0) & (vals < WCT), vals, -1)
                cadj[:, wi] = vals
            wi += 1
    out["celladjS"] = cadj.astype(BF16)

    NGRP, cover = plan["NGRP"], plan["cover"]
    KGMAX = plan["KGMAX"]
    qmt = np.zeros((NGRP, KGMAX, P, P), np.float32)
    for g in range(NGRP):
        off = 0
        for t in range(4):
            ch = g * 4 + t
            vals = cell[ch * P:(ch + 1) * P]
            for cw in cover[ch]:
                rv = vals - cw * WCW
                ok = (rv >= 0) & (rv < WCW)
                # matchT layout: [cell-rel partition, node]
                qmt[g, off, rv[ok], np.nonzero(ok)[0]] = 1.0
                off += 1
    out["qmats"] = qmt.astype(BF16)
    return out


def _build(plan, Mv):
    import concourse.bacc as bacc
    import concourse.mybir as mybir
    from concourse.tile import TileContext

    NN, NGRP = plan["NN"], plan["NGRP"]
    NCHUNKS, NSGS = plan["NCHUNKS"], plan["NSGS"]
    CT, W_OUT, CTT = plan["CT"], plan["W_OUT"], plan["CTT"]
    ch_lo_s, nwin_s, cover = plan["ch_lo_s"], plan["nwin_s"], plan["cover"]
    KMAX = plan["KMAX"]
    KGMAX = plan["KGMAX"]
    NWIN = int(nwin_s.sum())
    NWMAX = int(nwin_s.max())
    NIDX = SG * BUNDLE
    NCOL = (SG // P) * BUNDLE        # 32 gather columns per supergroup
    TT = SG // P                     # 8 node-subtiles per supergroup

    nc = bacc.Bacc("TRN2", target_bir_lowering=False, debug=False,
                   num_swdge_queues=4)
    dt = mybir.dt
    AL = mybir.AluOpType
    vtab4 = nc.declare_dram_parameter("vtab4", [Mv // 4, BUNDLE * CV], dt.bfloat16, isOutput=False)
    rtabsT_d = nc.declare_dram_parameter("rtabsT", [CR, W_OUT], dt.bfloat16, isOutput=False)
    vidx_d = nc.declare_dram_parameter("vidx", [P, NSGS, NIDX // 16], dt.int16, isOutput=False)
    qmaskf_d = nc.declare_dram_parameter("qmaskf", [P, NSGS, 4, NCOL], dt.bfloat16, isOutput=False)
    celladjS_d = nc.declare_dram_parameter("celladjS", [P, NWIN], dt.bfloat16, isOutput=False)
    qmats_d = nc.declare_dram_parameter("qmats", [NN // GRP, KGMAX, P, P], dt.bfloat16, isOutput=False)
    a16_d = nc.declare_dram_parameter("a16", [CR, E], dt.bfloat16, isOutput=False)
    wov_d = nc.declare_dram_parameter("wovT", [E, CO], dt.bfloat16, isOutput=False)
    out_d = nc.declare_dram_parameter("out", [CO, W_OUT], dt.float32, isOutput=True)

    with TileContext(nc) as tc:
        with (
            tc.tile_pool(name="res", bufs=1) as res,
            tc.tile_pool(name="x4p", bufs=4) as x4p,
            tc.tile_pool(name="mkp", bufs=2) as mkp,
            tc.tile_pool(name="xp", bufs=3) as xp,
            tc.tile_pool(name="small", bufs=2) as small,
            tc.tile_pool(name="scmp", bufs=3) as scmp,
            tc.tile_pool(name="prodp", bufs=3) as prodp,
            tc.tile_pool(name="xbp", bufs=1) as xbp,
            tc.tile_pool(name="scp", bufs=3) as scp,
            tc.tile_pool(name="psB", bufs=3, space="PSUM") as psB,
            tc.tile_pool(name="psC", bufs=2, space="PSUM") as psC,
            tc.tile_pool(name="psD", bufs=2, space="PSUM") as psD,
        ):
            # ---------- resident loads / constants ----------
            vixall = res.tile([P, NSGS, NIDX // 16], dt.int16)
            nc.sync.dma_start(out=vixall[:], in_=vidx_d[:])
            qmaskf = res.tile([P, NSGS, 4, NCOL], dt.bfloat16)
            nc.sync.dma_start(out=qmaskf[:, 0, :, :], in_=qmaskf_d[:, 0])
            nc.sync.dma_start(out=qmaskf[:, 1:, :, :], in_=qmaskf_d[:, 1:])
            celladjS = res.tile([P, NWIN], dt.bfloat16)
            nc.sync.dma_start(out=celladjS[:], in_=celladjS_d[:])
            a16 = res.tile([CR, E], dt.bfloat16)
            nc.sync.dma_start(out=a16[:], in_=a16_d[:])
            wovT = res.tile([E, CO], dt.bfloat16)
            nc.sync.dma_start(out=wovT[:], in_=wov_d[:])
            iotaRow16 = res.tile([P, WCT], dt.int16)
            nc.gpsimd.iota(iotaRow16[:], pattern=[[1, WCT]], base=0,
                           channel_multiplier=0)
            iotaRow = res.tile([P, WCT], dt.bfloat16)
            nc.scalar.copy(out=iotaRow[:], in_=iotaRow16[:])
            qtable = res.tile([P, CTT, E], dt.bfloat16)
            xbar_g = [xbp.tile([P, TT, E], dt.bfloat16, tag=f"xb{g}",
                                 name=f"xbar{g}")
                      for g in range(NSGS)]

            # ---------- q'-table: qtable[cell,:] = rtabsT[:,cell]^T @ A16 ----
            # built lazily, interleaved with the supergroup loop so the
            # bulk build never sits ahead of sg0 on the PE stream
            qtable_built = [0]

            def build_qtable_to(cw_end):
                for cw in range(qtable_built[0], min(cw_end, CTT)):
                    rsl = small.tile([CR, P], dt.bfloat16, tag="rsl")
                    nc.sync.dma_start(out=rsl[:],
                                      in_=rtabsT_d[:, cw * P:(cw + 1) * P])
                    qp = psB.tile([P, 4, E], dt.float32, tag="psB")
                    nc.tensor.matmul(out=qp[:, 0, :], lhsT=rsl[:],
                                     rhs=a16[:], start=True, stop=True)
                    nc.scalar.copy(out=qtable[:, cw, :], in_=qp[:, 0, :])
                qtable_built[0] = max(qtable_built[0], min(cw_end, CTT))

            # ---------- scatter: one-hot gen on chip + PE accumulate ------
            win_start = np.concatenate(([0], np.cumsum(nwin_s)))

            scmm_ready = {}

            def gen_scmm(ct):
                nw = int(nwin_s[ct])
                wi = int(win_start[ct])
                scmm = scmp.tile([P, NWMAX, WCT], dt.bfloat16, tag="scmm",
                                  name=f"scmm{ct}")
                cax = scmp.tile([P, NWMAX, WCT], dt.bfloat16, tag="cax",
                                name=f"cax{ct}")
                nc.scalar.copy(
                    out=cax[:, 0:nw, :],
                    in_=celladjS[:, wi:wi + nw, None].to_broadcast(
                        [P, nw, WCT]))
                nc.vector.tensor_tensor(
                    out=scmm[:, 0:nw, :],
                    in0=cax[:, 0:nw, :],
                    in1=iotaRow[:, None, :].to_broadcast([P, nw, WCT]),
                    op=AL.is_equal)
                scmm_ready[ct] = scmm

            def emit_scatter(ct):
                nw = int(nwin_s[ct])
                if ct not in scmm_ready:
                    gen_scmm(ct)
                scmm = scmm_ready.pop(ct)
                t1 = psC.tile([CO, WCT], dt.float32, tag="psC", name=f"t1_{ct}")
                for wv in range(nw):
                    ch = min(int(ch_lo_s[ct]) + wv, NCHUNKS - 1)
                    nc.tensor.matmul(out=t1[:],
                                     lhsT=xbar_g[ch // TT][:, ch % TT, :],
                                     rhs=scmm[:, wv, :], start=(wv == 0),
                                     stop=(wv == nw - 1))
                t1s = small.tile([CO, WCT], dt.bfloat16, tag="t1s",
                                 name=f"t1s{ct}")
                nc.scalar.copy(out=t1s[:], in_=t1[:])
                ot = psD.tile([CO, WCT], dt.float32, tag="psD", name=f"ot{ct}")
                nc.tensor.matmul(out=ot[:], lhsT=wovT[:], rhs=t1s[:],
                                 start=True, stop=True)
                osb = small.tile([CO, WCT], dt.float32, tag="osb",
                                 name=f"osb{ct}")
                nc.scalar.copy(out=osb[:], in_=ot[:])
                nc.scalar.dma_start(out=out_d[:, ct * WCT:(ct + 1) * WCT],
                                    in_=osb[:])

            # scatter window ct is ready once every chunk it reads is computed
            ct_ready = []
            for ct in range(CT):
                mx = min(int(ch_lo_s[ct]) + int(nwin_s[ct]) - 1, NCHUNKS - 1)
                ct_ready.append(mx + 1)

            issued = {}

            def issue_gather(sg):
                x4 = x4p.tile([P, NCOL, BUNDLE * CV], dt.bfloat16,
                              tag="x4", name=f"x4_{sg}")
                nh = NIDX // 2
                for h in (0, 1):
                    nc.gpsimd.dma_gather(
                        out_ap=x4[:, h * (NCOL // 2):(h + 1) * (NCOL // 2), :],
                        in_ap=vtab4[:],
                        idxs_ap=vixall[:, sg, h * (nh // 16):(h + 1) * (nh // 16)],
                        num_idxs=nh, num_idxs_reg=nh, elem_size=BUNDLE * CV,
                        single_packet=False, queue_num=(2 * sg + h) % 4)
                issued[sg] = x4

            for sg in range(min(4, NSGS)):
                issue_gather(sg)
            mask_tiles = {}
            m0 = mkp.tile([P, NCOL, BUNDLE * CV], dt.bfloat16,
                          tag="mask4", name="mask4_0")
            for mh in (0, 1):
                mhs = slice(mh * (NCOL // 2), (mh + 1) * (NCOL // 2))
                nc.scalar.copy(
                    out=m0[:, mhs, :].rearrange("p k (q c) -> p q k c", q=4),
                    in_=qmaskf[:, 0, :, mhs, None].to_broadcast(
                        [P, 4, NCOL // 2, CV]))
            mask_tiles[0] = m0
            done_ct = 0
            for sg in range(NSGS):
                if sg not in issued:
                    issue_gather(sg)
                x4 = issued.pop(sg)
                need_cw = max(max(cover[ch]) for ch in
                              range(sg * TT, (sg + 1) * TT)) + 1
                build_qtable_to(need_cw + 8)

                # expand NEXT sg's quad mask first so the scalar engine has
                # it ready before that sg's select needs it
                if sg + 1 < NSGS:
                    nmask = mkp.tile([P, NCOL, BUNDLE * CV], dt.bfloat16,
                                     tag="mask4", name=f"mask4_{sg + 1}")
                    for mh in (0, 1):
                        mhs = slice(mh * (NCOL // 2), (mh + 1) * (NCOL // 2))
                        nc.scalar.copy(
                            out=nmask[:, mhs, :].rearrange(
                                "p k (q c) -> p q k c", q=4),
                            in_=qmaskf[:, sg + 1, :, mhs, None].to_broadcast(
                                [P, 4, NCOL // 2, CV]))
                    mask_tiles[sg + 1] = nmask
                mask4 = mask_tiles.pop(sg)
                # pre-generate scatter one-hots for windows completing soon
                nd = done_ct
                cd = (sg + 1) * TT
                while nd < CT and ct_ready[nd] <= cd:
                    gen_scmm(nd)
                    nd += 1
                xsel = mask4
                xsg = xp.tile([P, NCOL, CV], dt.bfloat16, tag="x",
                              name=f"xsg{sg}")
                HC = NCOL // 2
                for h in (0, 1):
                    xse = xsel[:, h * HC:(h + 1) * HC, :]
                    xsh = xsg[:, h * HC:(h + 1) * HC, :]
                    nc.vector.tensor_tensor(
                        out=xse, in0=x4[:, h * HC:(h + 1) * HC, :],
                        in1=xse, op=AL.mult)
                    nc.vector.tensor_tensor(out=xsh, in0=xse[:, :, 0:CV],
                                            in1=xse[:, :, CV:2 * CV],
                                            op=AL.add)
                    nc.vector.tensor_tensor(out=xsh, in0=xsh,
                                            in1=xse[:, :, 2 * CV:3 * CV],
                                            op=AL.add)
                    nc.vector.tensor_tensor(out=xsh, in0=xsh,
                                            in1=xse[:, :, 3 * CV:4 * CV],
                                            op=AL.add)

                ex = scp.tile([P, NCOL], dt.float32, tag="ex")
                for gs in range(2):
                    g = sg * 2 + gs
                    xv = xsg[:, gs * 16:(gs + 1) * 16, :]

                    qps4 = psB.tile([P, 4, E], dt.float32, tag="psB")
                    kg = sum(len(cover[g * 4 + t]) for t in range(4))
                    mtg = small.tile([P, KGMAX, P], dt.bfloat16, tag="mtg")
                    nc.sync.dma_start(
                        out=mtg[:, 0:kg, :],
                        in_=qmats_d[g, 0:kg].rearrange("k p n -> p k n"))
                    off = 0
                    for t in range(4):
                        ch = g * 4 + t
                        cvr = cover[ch]
                        for ci, cw in enumerate(cvr):
                            nc.tensor.matmul(out=qps4[:, t, :],
                                             lhsT=mtg[:, off + ci, :],
                                             rhs=qtable[:, cw, :],
                                             start=(ci == 0),
                                             stop=(ci == len(cvr) - 1))
                        off += len(cvr)
                    qps4b = scp.tile([P, 4, E], dt.bfloat16, tag="qps4b")
                    nc.scalar.copy(out=qps4b[:], in_=qps4[:])
                    # scores: per-point dot(x, q') over channels
                    prod = prodp.tile([P, 16, CV], dt.bfloat16, tag="prod")
                    nc.vector.tensor_tensor(
                        out=prod[:].rearrange("p (t j) c -> p t j c", t=4),
                        in0=xv.rearrange("p (t j) c -> p t j c", t=4),
                        in1=qps4b[:, :, None, :].to_broadcast([P, 4, 4, E]),
                        op=AL.mult)
                    sc = scp.tile([P, 16], dt.bfloat16, tag="sc")
                    with nc.allow_low_precision(reason="bf16 score reduce"):
                        nc.vector.tensor_reduce(out=sc[:], in_=prod[:],
                                                axis=mybir.AxisListType.X,
                                                op=AL.add)
                    nc.scalar.activation(out=ex[:, gs * 16:(gs + 1) * 16],
                                         in_=sc[:],
                                         func=mybir.ActivationFunctionType.Exp)

                # merged softmax tail + weighted sum at supergroup level
                den = scp.tile([P, TT], dt.float32, tag="den")
                nc.vector.tensor_reduce(
                    out=den[:],
                    in_=ex[:].rearrange("p (t j) -> p t j", t=TT),
                    axis=mybir.AxisListType.X, op=AL.add)
                rec = scp.tile([P, TT], dt.float32, tag="rec")
                nc.vector.reciprocal(out=rec[:], in_=den[:])
                attn = scp.tile([P, NCOL], dt.bfloat16, tag="attn")
                nc.vector.tensor_tensor(
                    out=attn[:].rearrange("p (t j) -> p t j", t=TT),
                    in0=ex[:].rearrange("p (t j) -> p t j", t=TT),
                    in1=rec[:, :, None].to_broadcast([P, TT, 4]),
                    op=AL.mult)
                prod2 = scp.tile([P, NCOL, CV], dt.bfloat16, tag="prod2")
                nc.vector.tensor_tensor(
                    out=prod2[:], in0=xsg[:],
                    in1=attn[:, :, None].to_broadcast([P, NCOL, CV]),
                    op=AL.mult)
                pj = prod2[:].rearrange("p (g t j) c -> p j g t c", g=2, t=4)
                xb = xbar_g[sg][:].rearrange("p (g t) c -> p g t c", g=2)
                nc.vector.tensor_tensor(out=xb, in0=pj[:, 0], in1=pj[:, 1],
                                        op=AL.add)
                nc.vector.tensor_tensor(out=xb, in0=xb, in1=pj[:, 2],
                                        op=AL.add)
                nc.vector.tensor_tensor(out=xb, in0=xb, in1=pj[:, 3],
                                        op=AL.add)

                chunks_done = (sg + 1) * TT
                while done_ct < CT and ct_ready[done_ct] <= chunks_done:
                    emit_scatter(done_ct)
                    done_ct += 1

            build_qtable_to(CTT)
            for ct in range(done_ct, CT):
                emit_scatter(ct)
    nc.compile()
    return nc


def _install_ntff_shim():
    try:
        import antenv.axon_hooks  # noqa
        return
    except ImportError:
        pass
    try:
        from trn_agent_boot.trn_boot import _ntff_profile_via_ctypes
        hook = _ntff_profile_via_ctypes('/opt/axon/libaxon_pjrt.so')
        mod = types.ModuleType("antenv.axon_hooks")
        mod.get_axon_ntff_profile_hook = lambda: hook
        mod.set_axon_ntff_profile_hook = lambda h: None
        import antenv
        antenv.axon_hooks = mod
        sys.modules["antenv.axon_hooks"] = mod
    except Exception:
        pass


def kernel(**inputs):
    v_feat = np.asarray(inputs["v_feat"], np.float32)
    r_feat = np.asarray(inputs["r_feat"], np.float32)
    Wq = np.asarray(inputs["Wq"], np.float32)
    Wk = np.asarray(inputs["Wk"], np.float32)
    Wv = np.asarray(inputs["Wv"], np.float32)
    Wo = np.asarray(inputs["Wo"], np.float32)
    v2p = np.asarray(inputs["v2p_ind"])
    r2p = np.asarray(inputs["r2p_ind"])
    Mv = v_feat.shape[2]
    Mr = r_feat.shape[2]

    plan = _plan(v2p, r2p)
    nc = _build(plan, Mv)

    A16 = (Wq.T @ Wk / np.sqrt(np.float32(E))).astype(BF16)
    WovT16 = np.ascontiguousarray((Wo @ Wv).T).astype(BF16)

    in_maps = []
    vtab_cache = {}
    for c in plan["cores"]:
        arr = _core_arrays(c, plan, v_feat, r_feat, vtab_cache)
        arr["a16"] = A16
        arr["wovT"] = WovT16
        in_maps.append(arr)

    from concourse.bass_utils import run_bass_kernel_spmd
    _install_ntff_shim()
    trace = bool(inputs.get("_trace", False))
    res = run_bass_kernel_spmd(nc, in_maps, core_ids=list(range(8)),
                               trace=trace)
    out = np.zeros((B, CO, Mr), np.float32)
    for ci, c in enumerate(plan["cores"]):
        o = res.results[ci]["out"]
        w = min(c["width"], plan["W_OUT"])
        out[c["b"], :, c["clo"]:c["clo"] + w] = o[:, :w]
    kernel.last_exec_time_ns = res.exec_time_ns
    return out


kernel.last_exec_time_ns = None


# revision 53
# speedup vs baseline: 1.1135x; 1.0027x over previous
"""Trainium2 Bass kernel for nn_AttentionBlock2 (gnn_message_passing).

8 NeuronCores, SPMD, no collectives:
  - 2 batches x 4 cores; within a batch, nodes sorted by r-cell and split
    into 4 contiguous cell ranges (disjoint output slices per core).
  - v-gather: dma_gather (SWDGE, 4 queues) of quad-packed bf16 rows
    (4 feature rows per 512B table row -> int16 indexable); on-chip 4-way
    select as one all-bf16 multiply + 3 adds (DVE 2x mode) against a
    scalar-engine-expanded quad mask.
  - Math refactor: q' = (Wq^T Wk / sqrt(E)) r ; output proj Wov = Wo@Wv
    applied after the scatter.
  - One-hot match matrices generated on chip (iota + is_equal) instead of
    streamed from HBM; q'-expansion matmuls accumulate straight into PSUM
    (t-major column layout, no per-t replication copies).
  - scatter-add: PE matmuls of xbar vs one-hot match tiles, PSUM-
    accumulated per 256-cell output window, interleaved with compute.
"""

import sys
import types
import numpy as np
import ml_dtypes

B = 2
CV = 64
CR = 20
E = 64
CO = 64
BUNDLE = 4
P = 128
PER_B = 4
SG = 1024            # nodes per dma_gather call (SG*4 = 4096 idx)
GRP = 512            # nodes per compute group
WCT = 256            # scatter window width (cells)
WCW = 128            # q'-expansion window width (cells)
NEG = -(10 ** 9)

BF16 = ml_dtypes.bfloat16


def _plan(v2p, r2p):
    """Data-dependent but core-uniform plan."""
    Nn = r2p.shape[1]
    cores = []
    for b in range(B):
        cells = r2p[b, :, 0].astype(np.int64)
        order = np.argsort(cells, kind="stable")
        sc = cells[order]
        bounds = []
        for k in range(1, PER_B):
            c = sc[k * Nn // PER_B]
            bounds.append(int(np.searchsorted(sc, c)))
        pb = [0] + bounds + [Nn]
        for pi in range(PER_B):
            lo, hi = pb[pi], pb[pi + 1]
            nodes = order[lo:hi]
            clo = int(sc[lo])
            cores.append(dict(b=b, nodes=nodes, clo=clo,
                              width=int(sc[hi - 1]) + 1 - clo))
    nmax = max(len(c["nodes"]) for c in cores)
    NGRP = -(-nmax // GRP)
    gps = SG // GRP
    if NGRP % gps:
        NGRP += gps - NGRP % gps
    NN = NGRP * GRP
    NCHUNKS = NN // P
    NSGS = NN // SG
    wmax = max(c["width"] for c in cores)
    CT = -(-wmax // WCT)
    W_OUT = CT * WCT
    CTT = W_OUT // WCW

    for c in cores:
        n = len(c["nodes"])
        c["n"] = n
        cr = np.full(NN, NEG, np.int64)
        cr[:n] = r2p[c["b"], c["nodes"], 0].astype(np.int64) - c["clo"]
        c["cell"] = cr
        vr = np.zeros((NN, BUNDLE), np.int64)
        vr[:n] = v2p[c["b"], :, 0].reshape(Nn, BUNDLE)[c["nodes"]]
        c["vrow"] = vr

    ch_lo_s = np.full(CT, 10 ** 9, np.int64)
    ch_hi_s = np.zeros(CT, np.int64)
    ch_lo_t = np.full(CTT, 10 ** 9, np.int64)
    ch_hi_t = np.zeros(CTT, np.int64)
    for c in cores:
        cr = c["cell"]
        valid = cr > NEG
        for W, lo_arr, hi_arr, CN in ((WCT, ch_lo_s, ch_hi_s, CT),
                                      (WCW, ch_lo_t, ch_hi_t, CTT)):
            w_of = np.where(valid, cr // W, -1)
            for wi in range(CN):
                idx = np.nonzero(w_of == wi)[0]
                if len(idx):
                    lo_arr[wi] = min(lo_arr[wi], idx[0] // P)
                    hi_arr[wi] = max(hi_arr[wi], idx[-1] // P + 1)
    ch_lo_s = np.where(ch_lo_s > ch_hi_s, 0, ch_lo_s)
    nwin_s = np.maximum(ch_hi_s - ch_lo_s, 1).astype(np.int64)
    ch_lo_t = np.where(ch_lo_t > ch_hi_t, 0, ch_lo_t)
    nwin_t = np.maximum(ch_hi_t - ch_lo_t, 1).astype(np.int64)

    cover = [[] for _ in range(NCHUNKS)]
    for cw in range(CTT):
        if ch_hi_t[cw] == 0:      # no core has nodes in this window
            continue
        for ch in range(int(ch_lo_t[cw]), int(ch_lo_t[cw] + nwin_t[cw])):
            if 0 <= ch < NCHUNKS:
                cover[ch].append(cw)
    for ch in range(NCHUNKS):
        if not cover[ch]:
            cover[ch].append(0)
        lo, hi = min(cover[ch]), max(cover[ch])
        cover[ch] = list(range(lo, hi + 1))

    KMAX = max(len(cv) for cv in cover)
    ngrp2 = NN // GRP
    KGMAX = max(sum(len(cover[g * 4 + t]) for t in range(4))
                for g in range(ngrp2))
    return dict(cores=cores, NN=NN, NGRP=NGRP, NCHUNKS=NCHUNKS, NSGS=NSGS,
                KGMAX=KGMAX,
                CT=CT, W_OUT=W_OUT, CTT=CTT,
                ch_lo_s=ch_lo_s, nwin_s=nwin_s, cover=cover, KMAX=KMAX)


def _core_arrays(c, plan, v_feat, r_feat, vtab_cache):
    NN, NSGS, CT, W_OUT = plan["NN"], plan["NSGS"], plan["CT"], plan["W_OUT"]
    NCHUNKS = plan["NCHUNKS"]
    b = c["b"]
    out = {}
    if b not in vtab_cache:
        vt = np.ascontiguousarray(v_feat[b].T).astype(BF16)   # [Mv, 64]
        vtab_cache[b] = np.ascontiguousarray(vt.reshape(-1, BUNDLE * CV))
    out["vtab4"] = vtab_cache[b]
    rtT = np.zeros((CR, W_OUT), np.float32)
    w = min(c["width"], W_OUT)
    rtT[:, :w] = r_feat[b][:, c["clo"]: c["clo"] + w]
    out["rtabsT"] = np.ascontiguousarray(rtT).astype(BF16)

    vr = c["vrow"]
    NIDX = SG * BUNDLE
    # position (sg, k*P + p): k = gs*16 + t*4 + j (t-major), node =
    # sg*SG + gs*GRP + t*P + p, bundle member j.
    nodes_all = np.arange(NN)
    gps = SG // GRP
    gs_of = (nodes_all // GRP) % gps
    t_of = (nodes_all % GRP) // P
    p_of = nodes_all % P
    sg_of = nodes_all // SG
    rows = vr                                   # [NN, 4]
    k_of = gs_of * 16 + t_of * 4                # [NN]
    vidx = np.zeros((NSGS, NIDX), np.int64)
    quad = np.zeros((NSGS, P, (SG // P) * 4), np.uint8)
    for j in range(BUNDLE):
        kj = k_of + j
        vidx[sg_of, kj * P + p_of] = rows[:, j] // 4
        quad[sg_of, p_of, kj] = rows[:, j] % 4
    assert vidx.max() < 32768, "v row index exceeds int16 quad range"
    viw = np.tile(vidx.reshape(NSGS, NIDX // 16, 16).transpose(0, 2, 1),
                  (1, 8, 1))                       # [NSGS, P, NIDX//16]
    out["vidx"] = np.ascontiguousarray(
        viw.transpose(1, 0, 2)).astype(np.int16)   # [P, NSGS, NIDX//16]

    qm = np.zeros((NSGS, 4, P, (SG // P) * 4), np.float32)
    for qi in (0, 1, 2, 3):
        qm[:, qi] = (quad == qi)
    out["qmaskf"] = np.ascontiguousarray(
        qm.transpose(2, 0, 1, 3)).astype(BF16)     # [P, NSGS, 4, GPS*16]

    # window-relative cell values for on-chip one-hot generation
    ch_lo_s, nwin_s = plan["ch_lo_s"], plan["nwin_s"]
    KMAX, cover = plan["KMAX"], plan["cover"]
    NWIN = int(nwin_s.sum())
    cell = c["cell"]
    cadj = np.full((P, NWIN), -1, np.int64)
    wi = 0
    for ct in range(CT):
        for wv in range(int(nwin_s[ct])):
            ch = int(ch_lo_s[ct]) + wv
            if ch < NCHUNKS:
                vals = cell[ch * P:(ch + 1) * P] - ct * WCT
                vals = np.where((vals >= 0) & (vals < WCT), vals, -1)
                cadj[:, wi] = vals
            wi += 1
    out["celladjS"] = cadj.astype(BF16)

    NGRP, cover = plan["NGRP"], plan["cover"]
    KGMAX = plan["KGMAX"]
    qmt = np.zeros((NGRP, KGMAX, P, P), np.float32)
    for g in range(NGRP):
        off = 0
        for t in range(4):
            ch = g * 4 + t
            vals = cell[ch * P:(ch + 1) * P]
            for cw in cover[ch]:
                rv = vals - cw * WCW
                ok = (rv >= 0) & (rv < WCW)
                # matchT layout: [cell-rel partition, node]
                qmt[g, off, rv[ok], np.nonzero(ok)[0]] = 1.0
                off += 1
    out["qmats"] = qmt.astype(BF16)
    return out


def _build(plan, Mv):
    import concourse.bacc as bacc
    import concourse.mybir as mybir
    from concourse.tile import TileContext

    NN, NGRP = plan["NN"], plan["NGRP"]
    NCHUNKS, NSGS = plan["NCHUNKS"], plan["NSGS"]
    CT, W_OUT, CTT = plan["CT"], plan["W_OUT"], plan["CTT"]
    ch_lo_s, nwin_s, cover = plan["ch_lo_s"], plan["nwin_s"], plan["cover"]
    KMAX = plan["KMAX"]
    KGMAX = plan["KGMAX"]
    NWIN = int(nwin_s.sum())
    NWMAX = int(nwin_s.max())
    NIDX = SG * BUNDLE
    NCOL = (SG // P) * BUNDLE        # 32 gather columns per supergroup
    TT = SG // P                     # 8 node-subtiles per supergroup

    nc = bacc.Bacc("TRN2", target_bir_lowering=False, debug=False,
                   num_swdge_queues=4)
    dt = mybir.dt
    AL = mybir.AluOpType
    vtab4 = nc.declare_dram_parameter("vtab4", [Mv // 4, BUNDLE * CV], dt.bfloat16, isOutput=False)
    rtabsT_d = nc.declare_dram_parameter("rtabsT", [CR, W_OUT], dt.bfloat16, isOutput=False)
    vidx_d = nc.declare_dram_parameter("vidx", [P, NSGS, NIDX // 16], dt.int16, isOutput=False)
    qmaskf_d = nc.declare_dram_parameter("qmaskf", [P, NSGS, 4, NCOL], dt.bfloat16, isOutput=False)
    celladjS_d = nc.declare_dram_parameter("celladjS", [P, NWIN], dt.bfloat16, isOutput=False)
    qmats_d = nc.declare_dram_parameter("qmats", [NN // GRP, KGMAX, P, P], dt.bfloat16, isOutput=False)
    a16_d = nc.declare_dram_parameter("a16", [CR, E], dt.bfloat16, isOutput=False)
    wov_d = nc.declare_dram_parameter("wovT", [E, CO], dt.bfloat16, isOutput=False)
    out_d = nc.declare_dram_parameter("out", [CO, W_OUT], dt.float32, isOutput=True)

    with TileContext(nc) as tc:
        with (
            tc.tile_pool(name="res", bufs=1) as res,
            tc.tile_pool(name="x4p", bufs=4) as x4p,
            tc.tile_pool(name="mkp", bufs=2) as mkp,
            tc.tile_pool(name="xp", bufs=3) as xp,
            tc.tile_pool(name="small", bufs=2) as small,
            tc.tile_pool(name="scmp", bufs=3) as scmp,
            tc.tile_pool(name="prodp", bufs=3) as prodp,
            tc.tile_pool(name="xbp", bufs=1) as xbp,
            tc.tile_pool(name="scp", bufs=3) as scp,
            tc.tile_pool(name="psB", bufs=3, space="PSUM") as psB,
            tc.tile_pool(name="psC", bufs=2, space="PSUM") as psC,
            tc.tile_pool(name="psD", bufs=2, space="PSUM") as psD,
        ):
            # ---------- resident loads / constants ----------
            vixall = res.tile([P, NSGS, NIDX // 16], dt.int16)
            nc.sync.dma_start(out=vixall[:], in_=vidx_d[:])
            qmaskf = res.tile([P, NSGS, 4, NCOL], dt.bfloat16)
            nc.sync.dma_start(out=qmaskf[:, 0, :, :], in_=qmaskf_d[:, 0])
            nc.sync.dma_start(out=qmaskf[:, 1:, :, :], in_=qmaskf_d[:, 1:])
            celladjS = res.tile([P, NWIN], dt.bfloat16)
            nc.sync.dma_start(out=celladjS[:], in_=celladjS_d[:])
            a16 = res.tile([CR, E], dt.bfloat16)
            nc.sync.dma_start(out=a16[:], in_=a16_d[:])
            wovT = res.tile([E, CO], dt.bfloat16)
            nc.sync.dma_start(out=wovT[:], in_=wov_d[:])
            iotaRow16 = res.tile([P, WCT], dt.int16)
            nc.gpsimd.iota(iotaRow16[:], pattern=[[1, WCT]], base=0,
                           channel_multiplier=0)
            iotaRow = res.tile([P, WCT], dt.bfloat16)
            nc.scalar.copy(out=iotaRow[:], in_=iotaRow16[:])
            qtable = res.tile([P, CTT, E], dt.bfloat16)
            xbar_g = [xbp.tile([P, TT, E], dt.bfloat16, tag=f"xb{g}",
                                 name=f"xbar{g}")
                      for g in range(NSGS)]

            # ---------- q'-table: qtable[cell,:] = rtabsT[:,cell]^T @ A16 ----
            # built lazily, interleaved with the supergroup loop so the
            # bulk build never sits ahead of sg0 on the PE stream
            qtable_built = [0]

            def build_qtable_to(cw_end):
                for cw in range(qtable_built[0], min(cw_end, CTT)):
                    rsl = small.tile([CR, P], dt.bfloat16, tag="rsl")
                    nc.sync.dma_start(out=rsl[:],
                                      in_=rtabsT_d[:, cw * P:(cw + 1) * P])
                    qp = psB.tile([P, 4, E], dt.float32, tag="psB")
                    nc.tensor.matmul(out=qp[:, 0, :], lhsT=rsl[:],
                                     rhs=a16[:], start=True, stop=True)
                    nc.scalar.copy(out=qtable[:, cw, :], in_=qp[:, 0, :])
                qtable_built[0] = max(qtable_built[0], min(cw_end, CTT))

            # ---------- scatter: one-hot gen on chip + PE accumulate ------
            win_start = np.concatenate(([0], np.cumsum(nwin_s)))

            scmm_ready = {}

            def gen_scmm(ct):
                nw = int(nwin_s[ct])
                wi = int(win_start[ct])
                scmm = scmp.tile([P, NWMAX, WCT], dt.bfloat16, tag="scmm",
                                  name=f"scmm{ct}")
                cax = scmp.tile([P, NWMAX, WCT], dt.bfloat16, tag="cax",
                                name=f"cax{ct}")
                nc.scalar.copy(
                    out=cax[:, 0:nw, :],
                    in_=celladjS[:, wi:wi + nw, None].to_broadcast(
                        [P, nw, WCT]))
                nc.vector.tensor_tensor(
                    out=scmm[:, 0:nw, :],
                    in0=cax[:, 0:nw, :],
                    in1=iotaRow[:, None, :].to_broadcast([P, nw, WCT]),
                    op=AL.is_equal)
                scmm_ready[ct] = scmm

            def emit_scatter(ct):
                nw = int(nwin_s[ct])
                if ct not in scmm_ready:
                    gen_scmm(ct)
                scmm = scmm_ready.pop(ct)
                t1 = psC.tile([CO, WCT], dt.float32, tag="psC", name=f"t1_{ct}")
                for wv in range(nw):
                    ch = min(int(ch_lo_s[ct]) + wv, NCHUNKS - 1)
                    nc.tensor.matmul(out=t1[:],
                                     lhsT=xbar_g[ch // TT][:, ch % TT, :],
                                     rhs=scmm[:, wv, :], start=(wv == 0),
                                     stop=(wv == nw - 1))
                t1s = small.tile([CO, WCT], dt.bfloat16, tag="t1s",
                                 name=f"t1s{ct}")
                nc.scalar.copy(out=t1s[:], in_=t1[:])
                ot = psD.tile([CO, WCT], dt.float32, tag="psD", name=f"ot{ct}")
                nc.tensor.matmul(out=ot[:], lhsT=wovT[:], rhs=t1s[:],
                                 start=True, stop=True)
                osb = small.tile([CO, WCT], dt.float32, tag="osb",
                                 name=f"osb{ct}")
                nc.scalar.copy(out=osb[:], in_=ot[:])
                nc.scalar.dma_start(out=out_d[:, ct * WCT:(ct + 1) * WCT],
                                    in_=osb[:])

            # scatter window ct is ready once every chunk it reads is computed
            ct_ready = []
            for ct in range(CT):
                mx = min(int(ch_lo_s[ct]) + int(nwin_s[ct]) - 1, NCHUNKS - 1)
                ct_ready.append(mx + 1)

            issued = {}

            def issue_gather(sg):
                x4 = x4p.tile([P, NCOL, BUNDLE * CV], dt.bfloat16,
                              tag="x4", name=f"x4_{sg}")
                nh = NIDX // 4
                hc = NCOL // 4
                for h in range(4):
                    nc.gpsimd.dma_gather(
                        out_ap=x4[:, h * hc:(h + 1) * hc, :],
                        in_ap=vtab4[:],
                        idxs_ap=vixall[:, sg, h * (nh // 16):(h + 1) * (nh // 16)],
                        num_idxs=nh, num_idxs_reg=nh, elem_size=BUNDLE * CV,
                        single_packet=False, queue_num=h)
                issued[sg] = x4

            for sg in range(min(4, NSGS)):
                issue_gather(sg)
            mask_tiles = {}
            m0 = mkp.tile([P, NCOL, BUNDLE * CV], dt.bfloat16,
                          tag="mask4", name="mask4_0")
            for mh in (0, 1):
                mhs = slice(mh * (NCOL // 2), (mh + 1) * (NCOL // 2))
                nc.scalar.copy(
                    out=m0[:, mhs, :].rearrange("p k (q c) -> p q k c", q=4),
                    in_=qmaskf[:, 0, :, mhs, None].to_broadcast(
                        [P, 4, NCOL // 2, CV]))
            mask_tiles[0] = m0
            done_ct = 0
            for sg in range(NSGS):
                if sg not in issued:
                    issue_gather(sg)
                x4 = issued.pop(sg)
                need_cw = max(max(cover[ch]) for ch in
                              range(sg * TT, (sg + 1) * TT)) + 1
                build_qtable_to(need_cw + 8)

                # expand NEXT sg's quad mask first so the scalar engine has
                # it ready before that sg's select needs it
                if sg + 1 < NSGS:
                    nmask = mkp.tile([P, NCOL, BUNDLE * CV], dt.bfloat16,
                                     tag="mask4", name=f"mask4_{sg + 1}")
                    for mh in (0, 1):
                        mhs = slice(mh * (NCOL // 2), (mh + 1) * (NCOL // 2))
                        nc.scalar.copy(
                            out=nmask[:, mhs, :].rearrange(
                                "p k (q c) -> p q k c", q=4),
                            in_=qmaskf[:, sg + 1, :, mhs, None].to_broadcast(
                                [P, 4, NCOL // 2, CV]))
                    mask_tiles[sg + 1] = nmask
                mask4 = mask_tiles.pop(sg)
                # pre-generate scatter one-hots for windows completing soon
                nd = done_ct
                cd = (sg + 1) * TT
                while nd < CT and ct_ready[nd] <= cd:
                    gen_scmm(nd)
                    nd += 1
                xsel = mask4
                xsg = xp.tile([P, NCOL, CV], dt.bfloat16, tag="x",
                              name=f"xsg{sg}")
                HC = NCOL // 4
                for h in range(4):
                    xse = xsel[:, h * HC:(h + 1) * HC, :]
                    xsh = xsg[:, h * HC:(h + 1) * HC, :]
                    nc.vector.tensor_tensor(
                        out=xse, in0=x4[:, h * HC:(h + 1) * HC, :],
                        in1=xse, op=AL.mult)
                    nc.vector.tensor_tensor(out=xsh, in0=xse[:, :, 0:CV],
                                            in1=xse[:, :, CV:2 * CV],
                                            op=AL.add)
                    nc.vector.tensor_tensor(out=xsh, in0=xsh,
                                            in1=xse[:, :, 2 * CV:3 * CV],
                                            op=AL.add)
                    nc.vector.tensor_tensor(out=xsh, in0=xsh,
                                            in1=xse[:, :, 3 * CV:4 * CV],
                                            op=AL.add)

                ex = scp.tile([P, NCOL], dt.float32, tag="ex")
                for gs in range(2):
                    g = sg * 2 + gs
                    xv = xsg[:, gs * 16:(gs + 1) * 16, :]

                    qps4 = psB.tile([P, 4, E], dt.float32, tag="psB")
                    kg = sum(len(cover[g * 4 + t]) for t in range(4))
                    mtg = small.tile([P, KGMAX, P], dt.bfloat16, tag="mtg")
                    nc.sync.dma_start(
                        out=mtg[:, 0:kg, :],
                        in_=qmats_d[g, 0:kg].rearrange("k p n -> p k n"))
                    off = 0
                    for t in range(4):
                        ch = g * 4 + t
                        cvr = cover[ch]
                        for ci, cw in enumerate(cvr):
                            nc.tensor.matmul(out=qps4[:, t, :],
                                             lhsT=mtg[:, off + ci, :],
                                             rhs=qtable[:, cw, :],
                                             start=(ci == 0),
                                             stop=(ci == len(cvr) - 1))
                        off += len(cvr)
                    qps4b = scp.tile([P, 4, E], dt.bfloat16, tag="qps4b")
                    nc.scalar.copy(out=qps4b[:], in_=qps4[:])
                    # scores: per-point dot(x, q') over channels
                    prod = prodp.tile([P, 16, CV], dt.bfloat16, tag="prod")
                    nc.vector.tensor_tensor(
                        out=prod[:].rearrange("p (t j) c -> p t j c", t=4),
                        in0=xv.rearrange("p (t j) c -> p t j c", t=4),
                        in1=qps4b[:, :, None, :].to_broadcast([P, 4, 4, E]),
                        op=AL.mult)
                    sc = scp.tile([P, 16], dt.bfloat16, tag="sc")
                    with nc.allow_low_precision(reason="bf16 score reduce"):
                        nc.vector.tensor_reduce(out=sc[:], in_=prod[:],
                                                axis=mybir.AxisListType.X,
                                                op=AL.add)
                    nc.scalar.activation(out=ex[:, gs * 16:(gs + 1) * 16],
                                         in_=sc[:],
                                         func=mybir.ActivationFunctionType.Exp)

                # merged softmax tail + weighted sum at supergroup level
                den = scp.tile([P, TT], dt.float32, tag="den")
                nc.vector.tensor_reduce(
                    out=den[:],
                    in_=ex[:].rearrange("p (t j) -> p t j", t=TT),
                    axis=mybir.AxisListType.X, op=AL.add)
                rec = scp.tile([P, TT], dt.float32, tag="rec")
                nc.vector.reciprocal(out=rec[:], in_=den[:])
                attn = scp.tile([P, NCOL], dt.bfloat16, tag="attn")
                nc.vector.tensor_tensor(
                    out=attn[:].rearrange("p (t j) -> p t j", t=TT),
                    in0=ex[:].rearrange("p (t j) -> p t j", t=TT),
                    in1=rec[:, :, None].to_broadcast([P, TT, 4]),
                    op=AL.mult)
                prod2 = scp.tile([P, NCOL, CV], dt.bfloat16, tag="prod2")
                nc.vector.tensor_tensor(
                    out=prod2[:], in0=xsg[:],
                    in1=attn[:, :, None].to_broadcast([P, NCOL, CV]),
                    op=AL.mult)
                pj = prod2[:].rearrange("p (g t j) c -> p j g t c", g=2, t=4)
                xb = xbar_g[sg][:].rearrange("p (g t) c -> p g t c", g=2)
                nc.vector.tensor_tensor(out=xb, in0=pj[:, 0], in1=pj[:, 1],
                                        op=AL.add)
                nc.vector.tensor_tensor(out=xb, in0=xb, in1=pj[:, 2],
                                        op=AL.add)
                nc.vector.tensor_tensor(out=xb, in0=xb, in1=pj[:, 3],
                                        op=AL.add)

                chunks_done = (sg + 1) * TT
                while done_ct < CT and ct_ready[done_ct] <= chunks_done:
                    emit_scatter(done_ct)
                    done_ct += 1

            build_qtable_to(CTT)
            for ct in range(done_ct, CT):
                emit_scatter(ct)
    nc.compile()
    return nc


def _install_ntff_shim():
    try:
        import antenv.axon_hooks  # noqa
        return
    except ImportError:
        pass
    try:
        from trn_agent_boot.trn_boot import _ntff_profile_via_ctypes
        hook = _ntff_profile_via_ctypes('/opt/axon/libaxon_pjrt.so')
        mod = types.ModuleType("antenv.axon_hooks")
        mod.get_axon_ntff_profile_hook = lambda: hook
        mod.set_axon_ntff_profile_hook = lambda h: None
        import antenv
        antenv.axon_hooks = mod
        sys.modules["antenv.axon_hooks"] = mod
    except Exception:
        pass


def kernel(**inputs):
    v_feat = np.asarray(inputs["v_feat"], np.float32)
    r_feat = np.asarray(inputs["r_feat"], np.float32)
    Wq = np.asarray(inputs["Wq"], np.float32)
    Wk = np.asarray(inputs["Wk"], np.float32)
    Wv = np.asarray(inputs["Wv"], np.float32)
    Wo = np.asarray(inputs["Wo"], np.float32)
    v2p = np.asarray(inputs["v2p_ind"])
    r2p = np.asarray(inputs["r2p_ind"])
    Mv = v_feat.shape[2]
    Mr = r_feat.shape[2]

    plan = _plan(v2p, r2p)
    nc = _build(plan, Mv)

    A16 = (Wq.T @ Wk / np.sqrt(np.float32(E))).astype(BF16)
    WovT16 = np.ascontiguousarray((Wo @ Wv).T).astype(BF16)

    in_maps = []
    vtab_cache = {}
    for c in plan["cores"]:
        arr = _core_arrays(c, plan, v_feat, r_feat, vtab_cache)
        arr["a16"] = A16
        arr["wovT"] = WovT16
        in_maps.append(arr)

    from concourse.bass_utils import run_bass_kernel_spmd
    _install_ntff_shim()
    trace = bool(inputs.get("_trace", False))
    res = run_bass_kernel_spmd(nc, in_maps, core_ids=list(range(8)),
                               trace=trace)
    out = np.zeros((B, CO, Mr), np.float32)
    for ci, c in enumerate(plan["cores"]):
        o = res.results[ci]["out"]
        w = min(c["width"], plan["W_OUT"])
        out[c["b"], :, c["clo"]:c["clo"] + w] = o[:, :w]
    kernel.last_exec_time_ns = res.exec_time_ns
    return out


kernel.last_exec_time_ns = None


# revision 54
# speedup vs baseline: 1.1575x; 1.0395x over previous
"""Trainium2 Bass kernel for nn_AttentionBlock2 (gnn_message_passing).

8 NeuronCores, SPMD, no collectives:
  - 2 batches x 4 cores; within a batch, nodes sorted by r-cell and split
    into 4 contiguous cell ranges (disjoint output slices per core).
  - v-gather: dma_gather (SWDGE, 4 queues) of quad-packed bf16 rows
    (4 feature rows per 512B table row -> int16 indexable); on-chip 4-way
    select as one all-bf16 multiply + 3 adds (DVE 2x mode) against a
    scalar-engine-expanded quad mask.
  - Math refactor: q' = (Wq^T Wk / sqrt(E)) r ; output proj Wov = Wo@Wv
    applied after the scatter.
  - One-hot match matrices generated on chip (iota + is_equal) instead of
    streamed from HBM; q'-expansion matmuls accumulate straight into PSUM
    (t-major column layout, no per-t replication copies).
  - scatter-add: PE matmuls of xbar vs one-hot match tiles, PSUM-
    accumulated per 256-cell output window, interleaved with compute.
"""

import sys
import types
import numpy as np
import ml_dtypes

B = 2
CV = 64
CR = 20
E = 64
CO = 64
BUNDLE = 4
P = 128
PER_B = 4
SG = 1024            # nodes per dma_gather call (SG*4 = 4096 idx)
GRP = 512            # nodes per compute group
WCT = 256            # scatter window width (cells)
WCW = 128            # q'-expansion window width (cells)
NEG = -(10 ** 9)

BF16 = ml_dtypes.bfloat16


def _plan(v2p, r2p):
    """Data-dependent but core-uniform plan."""
    Nn = r2p.shape[1]
    cores = []
    for b in range(B):
        cells = r2p[b, :, 0].astype(np.int64)
        order = np.argsort(cells, kind="stable")
        sc = cells[order]
        bounds = []
        for k in range(1, PER_B):
            c = sc[k * Nn // PER_B]
            bounds.append(int(np.searchsorted(sc, c)))
        pb = [0] + bounds + [Nn]
        for pi in range(PER_B):
            lo, hi = pb[pi], pb[pi + 1]
            nodes = order[lo:hi]
            clo = int(sc[lo])
            cores.append(dict(b=b, nodes=nodes, clo=clo,
                              width=int(sc[hi - 1]) + 1 - clo))
    nmax = max(len(c["nodes"]) for c in cores)
    NGRP = -(-nmax // GRP)
    gps = SG // GRP
    if NGRP % gps:
        NGRP += gps - NGRP % gps
    NN = NGRP * GRP
    NCHUNKS = NN // P
    NSGS = NN // SG
    wmax = max(c["width"] for c in cores)
    CT = -(-wmax // WCT)
    W_OUT = CT * WCT
    CTT = W_OUT // WCW

    for c in cores:
        n = len(c["nodes"])
        c["n"] = n
        cr = np.full(NN, NEG, np.int64)
        cr[:n] = r2p[c["b"], c["nodes"], 0].astype(np.int64) - c["clo"]
        c["cell"] = cr
        vr = np.zeros((NN, BUNDLE), np.int64)
        vr[:n] = v2p[c["b"], :, 0].reshape(Nn, BUNDLE)[c["nodes"]]
        c["vrow"] = vr

    ch_lo_s = np.full(CT, 10 ** 9, np.int64)
    ch_hi_s = np.zeros(CT, np.int64)
    ch_lo_t = np.full(CTT, 10 ** 9, np.int64)
    ch_hi_t = np.zeros(CTT, np.int64)
    for c in cores:
        cr = c["cell"]
        valid = cr > NEG
        for W, lo_arr, hi_arr, CN in ((WCT, ch_lo_s, ch_hi_s, CT),
                                      (WCW, ch_lo_t, ch_hi_t, CTT)):
            w_of = np.where(valid, cr // W, -1)
            for wi in range(CN):
                idx = np.nonzero(w_of == wi)[0]
                if len(idx):
                    lo_arr[wi] = min(lo_arr[wi], idx[0] // P)
                    hi_arr[wi] = max(hi_arr[wi], idx[-1] // P + 1)
    ch_lo_s = np.where(ch_lo_s > ch_hi_s, 0, ch_lo_s)
    nwin_s = np.maximum(ch_hi_s - ch_lo_s, 1).astype(np.int64)
    ch_lo_t = np.where(ch_lo_t > ch_hi_t, 0, ch_lo_t)
    nwin_t = np.maximum(ch_hi_t - ch_lo_t, 1).astype(np.int64)

    cover = [[] for _ in range(NCHUNKS)]
    for cw in range(CTT):
        if ch_hi_t[cw] == 0:      # no core has nodes in this window
            continue
        for ch in range(int(ch_lo_t[cw]), int(ch_lo_t[cw] + nwin_t[cw])):
            if 0 <= ch < NCHUNKS:
                cover[ch].append(cw)
    for ch in range(NCHUNKS):
        if not cover[ch]:
            cover[ch].append(0)
        lo, hi = min(cover[ch]), max(cover[ch])
        cover[ch] = list(range(lo, hi + 1))

    KMAX = max(len(cv) for cv in cover)
    ngrp2 = NN // GRP
    KGMAX = max(sum(len(cover[g * 4 + t]) for t in range(4))
                for g in range(ngrp2))
    return dict(cores=cores, NN=NN, NGRP=NGRP, NCHUNKS=NCHUNKS, NSGS=NSGS,
                KGMAX=KGMAX,
                CT=CT, W_OUT=W_OUT, CTT=CTT,
                ch_lo_s=ch_lo_s, nwin_s=nwin_s, cover=cover, KMAX=KMAX)


def _core_arrays(c, plan, v_feat, r_feat, vtab_cache):
    NN, NSGS, CT, W_OUT = plan["NN"], plan["NSGS"], plan["CT"], plan["W_OUT"]
    NCHUNKS = plan["NCHUNKS"]
    b = c["b"]
    out = {}
    if b not in vtab_cache:
        vt = np.ascontiguousarray(v_feat[b].T).astype(BF16)   # [Mv, 64]
        vtab_cache[b] = np.ascontiguousarray(vt.reshape(-1, BUNDLE * CV))
    out["vtab4"] = vtab_cache[b]
    rtT = np.zeros((CR, W_OUT), np.float32)
    w = min(c["width"], W_OUT)
    rtT[:, :w] = r_feat[b][:, c["clo"]: c["clo"] + w]
    out["rtabsT"] = np.ascontiguousarray(rtT).astype(BF16)

    vr = c["vrow"]
    NIDX = SG * BUNDLE
    # position (sg, k*P + p): k = gs*16 + t*4 + j (t-major), node =
    # sg*SG + gs*GRP + t*P + p, bundle member j.
    nodes_all = np.arange(NN)
    gps = SG // GRP
    gs_of = (nodes_all // GRP) % gps
    t_of = (nodes_all % GRP) // P
    p_of = nodes_all % P
    sg_of = nodes_all // SG
    rows = vr                                   # [NN, 4]
    k_of = gs_of * 16 + t_of * 4                # [NN]
    vidx = np.zeros((NSGS, NIDX), np.int64)
    quad = np.zeros((NSGS, P, (SG // P) * 4), np.uint8)
    for j in range(BUNDLE):
        kj = k_of + j
        vidx[sg_of, kj * P + p_of] = rows[:, j] // 4
        quad[sg_of, p_of, kj] = rows[:, j] % 4
    assert vidx.max() < 32768, "v row index exceeds int16 quad range"
    viw = np.tile(vidx.reshape(NSGS, NIDX // 16, 16).transpose(0, 2, 1),
                  (1, 8, 1))                       # [NSGS, P, NIDX//16]
    out["vidx"] = np.ascontiguousarray(
        viw.transpose(1, 0, 2)).astype(np.int16)   # [P, NSGS, NIDX//16]

    qm = np.zeros((NSGS, 4, P, (SG // P) * 4), np.float32)
    for qi in (0, 1, 2, 3):
        qm[:, qi] = (quad == qi)
    out["qmaskf"] = np.ascontiguousarray(
        qm.transpose(2, 0, 1, 3)).astype(BF16)     # [P, NSGS, 4, GPS*16]

    # window-relative cell values for on-chip one-hot generation
    ch_lo_s, nwin_s = plan["ch_lo_s"], plan["nwin_s"]
    KMAX, cover = plan["KMAX"], plan["cover"]
    NWIN = int(nwin_s.sum())
    cell = c["cell"]
    cadj = np.full((P, NWIN), -1, np.int64)
    wi = 0
    for ct in range(CT):
        for wv in range(int(nwin_s[ct])):
            ch = int(ch_lo_s[ct]) + wv
            if ch < NCHUNKS:
                vals = cell[ch * P:(ch + 1) * P] - ct * WCT
                vals = np.where((vals >= 0) & (vals < WCT), vals, -1)
                cadj[:, wi] = vals
            wi += 1
    out["celladjS"] = cadj.astype(BF16)

    NGRP, cover = plan["NGRP"], plan["cover"]
    KGMAX = plan["KGMAX"]
    qmt = np.zeros((NGRP, KGMAX, P, P), np.float32)
    for g in range(NGRP):
        off = 0
        for t in range(4):
            ch = g * 4 + t
            vals = cell[ch * P:(ch + 1) * P]
            for cw in cover[ch]:
                rv = vals - cw * WCW
                ok = (rv >= 0) & (rv < WCW)
                # matchT layout: [cell-rel partition, node]
                qmt[g, off, rv[ok], np.nonzero(ok)[0]] = 1.0
                off += 1
    out["qmats"] = qmt.astype(BF16)
    return out


def _build(plan, Mv):
    import concourse.bacc as bacc
    import concourse.mybir as mybir
    from concourse.tile import TileContext

    NN, NGRP = plan["NN"], plan["NGRP"]
    NCHUNKS, NSGS = plan["NCHUNKS"], plan["NSGS"]
    CT, W_OUT, CTT = plan["CT"], plan["W_OUT"], plan["CTT"]
    ch_lo_s, nwin_s, cover = plan["ch_lo_s"], plan["nwin_s"], plan["cover"]
    KMAX = plan["KMAX"]
    KGMAX = plan["KGMAX"]
    NWIN = int(nwin_s.sum())
    NWMAX = int(nwin_s.max())
    NIDX = SG * BUNDLE
    NCOL = (SG // P) * BUNDLE        # 32 gather columns per supergroup
    TT = SG // P                     # 8 node-subtiles per supergroup

    nc = bacc.Bacc("TRN2", target_bir_lowering=False, debug=False,
                   num_swdge_queues=4)
    dt = mybir.dt
    AL = mybir.AluOpType
    vtab4 = nc.declare_dram_parameter("vtab4", [Mv // 4, BUNDLE * CV], dt.bfloat16, isOutput=False)
    rtabsT_d = nc.declare_dram_parameter("rtabsT", [CR, W_OUT], dt.bfloat16, isOutput=False)
    vidx_d = nc.declare_dram_parameter("vidx", [P, NSGS, NIDX // 16], dt.int16, isOutput=False)
    qmaskf_d = nc.declare_dram_parameter("qmaskf", [P, NSGS, 4, NCOL], dt.bfloat16, isOutput=False)
    celladjS_d = nc.declare_dram_parameter("celladjS", [P, NWIN], dt.bfloat16, isOutput=False)
    qmats_d = nc.declare_dram_parameter("qmats", [NN // GRP, KGMAX, P, P], dt.bfloat16, isOutput=False)
    a16_d = nc.declare_dram_parameter("a16", [CR, E], dt.bfloat16, isOutput=False)
    wov_d = nc.declare_dram_parameter("wovT", [E, CO], dt.bfloat16, isOutput=False)
    out_d = nc.declare_dram_parameter("out", [CO, W_OUT], dt.float32, isOutput=True)

    with TileContext(nc) as tc:
        with (
            tc.tile_pool(name="res", bufs=1) as res,
            tc.tile_pool(name="x4p", bufs=3) as x4p,
            tc.tile_pool(name="mkp", bufs=2) as mkp,
            tc.tile_pool(name="xp", bufs=3) as xp,
            tc.tile_pool(name="small", bufs=2) as small,
            tc.tile_pool(name="scmp", bufs=3) as scmp,
            tc.tile_pool(name="prodp", bufs=3) as prodp,
            tc.tile_pool(name="xbp", bufs=1) as xbp,
            tc.tile_pool(name="scp", bufs=3) as scp,
            tc.tile_pool(name="psB", bufs=3, space="PSUM") as psB,
            tc.tile_pool(name="psC", bufs=2, space="PSUM") as psC,
            tc.tile_pool(name="psD", bufs=2, space="PSUM") as psD,
        ):
            # ---------- resident loads / constants ----------
            vixall = res.tile([P, NSGS, NIDX // 16], dt.int16)
            nc.sync.dma_start(out=vixall[:], in_=vidx_d[:])
            qmaskf = res.tile([P, NSGS, 4, NCOL], dt.bfloat16)
            nc.sync.dma_start(out=qmaskf[:, 0, :, :], in_=qmaskf_d[:, 0])
            nc.sync.dma_start(out=qmaskf[:, 1:, :, :], in_=qmaskf_d[:, 1:])
            celladjS = res.tile([P, NWIN], dt.bfloat16)
            nc.sync.dma_start(out=celladjS[:], in_=celladjS_d[:])
            a16 = res.tile([CR, E], dt.bfloat16)
            nc.sync.dma_start(out=a16[:], in_=a16_d[:])
            wovT = res.tile([E, CO], dt.bfloat16)
            nc.sync.dma_start(out=wovT[:], in_=wov_d[:])
            iotaRow16 = res.tile([P, WCT], dt.int16)
            nc.gpsimd.iota(iotaRow16[:], pattern=[[1, WCT]], base=0,
                           channel_multiplier=0)
            iotaRow = res.tile([P, WCT], dt.bfloat16)
            nc.scalar.copy(out=iotaRow[:], in_=iotaRow16[:])
            qtable = res.tile([P, CTT, E], dt.bfloat16)
            xbar_g = [xbp.tile([P, TT, E], dt.bfloat16, tag=f"xb{g}",
                                 name=f"xbar{g}")
                      for g in range(NSGS)]

            # ---------- q'-table: qtable[cell,:] = rtabsT[:,cell]^T @ A16 ----
            # built lazily, interleaved with the supergroup loop so the
            # bulk build never sits ahead of sg0 on the PE stream
            qtable_built = [0]

            def build_qtable_to(cw_end):
                for cw in range(qtable_built[0], min(cw_end, CTT)):
                    rsl = small.tile([CR, P], dt.bfloat16, tag="rsl")
                    nc.sync.dma_start(out=rsl[:],
                                      in_=rtabsT_d[:, cw * P:(cw + 1) * P])
                    qp = psB.tile([P, 4, E], dt.float32, tag="psB")
                    nc.tensor.matmul(out=qp[:, 0, :], lhsT=rsl[:],
                                     rhs=a16[:], start=True, stop=True)
                    nc.scalar.copy(out=qtable[:, cw, :], in_=qp[:, 0, :])
                qtable_built[0] = max(qtable_built[0], min(cw_end, CTT))

            # ---------- scatter: one-hot gen on chip + PE accumulate ------
            win_start = np.concatenate(([0], np.cumsum(nwin_s)))

            scmm_ready = {}

            def gen_scmm(ct):
                nw = int(nwin_s[ct])
                wi = int(win_start[ct])
                scmm = scmp.tile([P, NWMAX, WCT], dt.bfloat16, tag="scmm",
                                  name=f"scmm{ct}")
                cax = scmp.tile([P, NWMAX, WCT], dt.bfloat16, tag="cax",
                                name=f"cax{ct}")
                nc.scalar.copy(
                    out=cax[:, 0:nw, :],
                    in_=celladjS[:, wi:wi + nw, None].to_broadcast(
                        [P, nw, WCT]))
                nc.vector.tensor_tensor(
                    out=scmm[:, 0:nw, :],
                    in0=cax[:, 0:nw, :],
                    in1=iotaRow[:, None, :].to_broadcast([P, nw, WCT]),
                    op=AL.is_equal)
                scmm_ready[ct] = scmm

            def emit_scatter(ct):
                nw = int(nwin_s[ct])
                if ct not in scmm_ready:
                    gen_scmm(ct)
                scmm = scmm_ready.pop(ct)
                t1 = psC.tile([CO, WCT], dt.float32, tag="psC", name=f"t1_{ct}")
                for wv in range(nw):
                    ch = min(int(ch_lo_s[ct]) + wv, NCHUNKS - 1)
                    nc.tensor.matmul(out=t1[:],
                                     lhsT=xbar_g[ch // TT][:, ch % TT, :],
                                     rhs=scmm[:, wv, :], start=(wv == 0),
                                     stop=(wv == nw - 1))
                t1s = small.tile([CO, WCT], dt.bfloat16, tag="t1s",
                                 name=f"t1s{ct}")
                nc.scalar.copy(out=t1s[:], in_=t1[:])
                ot = psD.tile([CO, WCT], dt.float32, tag="psD", name=f"ot{ct}")
                nc.tensor.matmul(out=ot[:], lhsT=wovT[:], rhs=t1s[:],
                                 start=True, stop=True)
                osb = small.tile([CO, WCT], dt.float32, tag="osb",
                                 name=f"osb{ct}")
                nc.scalar.copy(out=osb[:], in_=ot[:])
                nc.scalar.dma_start(out=out_d[:, ct * WCT:(ct + 1) * WCT],
                                    in_=osb[:])

            # scatter window ct is ready once every chunk it reads is computed
            ct_ready = []
            for ct in range(CT):
                mx = min(int(ch_lo_s[ct]) + int(nwin_s[ct]) - 1, NCHUNKS - 1)
                ct_ready.append(mx + 1)

            issued = {}

            def issue_gather(sg):
                x4 = x4p.tile([P, NCOL, BUNDLE * CV], dt.bfloat16,
                              tag="x4", name=f"x4_{sg}")
                nh = NIDX // 4
                hc = NCOL // 4
                for h in range(4):
                    nc.gpsimd.dma_gather(
                        out_ap=x4[:, h * hc:(h + 1) * hc, :],
                        in_ap=vtab4[:],
                        idxs_ap=vixall[:, sg, h * (nh // 16):(h + 1) * (nh // 16)],
                        num_idxs=nh, num_idxs_reg=nh, elem_size=BUNDLE * CV,
                        single_packet=False, queue_num=h)
                issued[sg] = x4

            for sg in range(min(3, NSGS)):
                issue_gather(sg)
            mask_tiles = {}
            m0 = mkp.tile([P, NCOL, BUNDLE * CV], dt.bfloat16,
                          tag="mask4", name="mask4_0")
            for mh in (0, 1):
                mhs = slice(mh * (NCOL // 2), (mh + 1) * (NCOL // 2))
                nc.scalar.copy(
                    out=m0[:, mhs, :].rearrange("p k (q c) -> p q k c", q=4),
                    in_=qmaskf[:, 0, :, mhs, None].to_broadcast(
                        [P, 4, NCOL // 2, CV]))
            mask_tiles[0] = m0
            done_ct = 0
            for sg in range(NSGS):
                if sg not in issued:
                    issue_gather(sg)
                x4 = issued.pop(sg)
                need_cw = max(max(cover[ch]) for ch in
                              range(sg * TT, (sg + 1) * TT)) + 1
                build_qtable_to(need_cw + 8)

                # expand NEXT sg's quad mask first so the scalar engine has
                # it ready before that sg's select needs it
                if sg + 1 < NSGS:
                    nmask = mkp.tile([P, NCOL, BUNDLE * CV], dt.bfloat16,
                                     tag="mask4", name=f"mask4_{sg + 1}")
                    for mh in (0, 1):
                        mhs = slice(mh * (NCOL // 2), (mh + 1) * (NCOL // 2))
                        nc.scalar.copy(
                            out=nmask[:, mhs, :].rearrange(
                                "p k (q c) -> p q k c", q=4),
                            in_=qmaskf[:, sg + 1, :, mhs, None].to_broadcast(
                                [P, 4, NCOL // 2, CV]))
                    mask_tiles[sg + 1] = nmask
                mask4 = mask_tiles.pop(sg)
                # pre-generate scatter one-hots for windows completing soon
                nd = done_ct
                cd = (sg + 1) * TT
                while nd < CT and ct_ready[nd] <= cd:
                    gen_scmm(nd)
                    nd += 1
                xsel = mask4
                xsg = xp.tile([P, NCOL, CV], dt.bfloat16, tag="x",
                              name=f"xsg{sg}")
                HC = NCOL // 4
                for h in range(4):
                    xse = xsel[:, h * HC:(h + 1) * HC, :]
                    xsh = xsg[:, h * HC:(h + 1) * HC, :]
                    nc.vector.tensor_tensor(
                        out=xse, in0=x4[:, h * HC:(h + 1) * HC, :],
                        in1=xse, op=AL.mult)
                    nc.vector.tensor_tensor(out=xsh, in0=xse[:, :, 0:CV],
                                            in1=xse[:, :, CV:2 * CV],
                                            op=AL.add)
                    nc.vector.tensor_tensor(out=xsh, in0=xsh,
                                            in1=xse[:, :, 2 * CV:3 * CV],
                                            op=AL.add)
                    nc.vector.tensor_tensor(out=xsh, in0=xsh,
                                            in1=xse[:, :, 3 * CV:4 * CV],
                                            op=AL.add)

                ex = scp.tile([P, NCOL], dt.float32, tag="ex")
                for gs in range(2):
                    g = sg * 2 + gs
                    xv = xsg[:, gs * 16:(gs + 1) * 16, :]

                    qps4 = psB.tile([P, 4, E], dt.float32, tag="psB")
                    kg = sum(len(cover[g * 4 + t]) for t in range(4))
                    mtg = small.tile([P, KGMAX, P], dt.bfloat16, tag="mtg")
                    nc.sync.dma_start(
                        out=mtg[:, 0:kg, :],
                        in_=qmats_d[g, 0:kg].rearrange("k p n -> p k n"))
                    off = 0
                    for t in range(4):
                        ch = g * 4 + t
                        cvr = cover[ch]
                        for ci, cw in enumerate(cvr):
                            nc.tensor.matmul(out=qps4[:, t, :],
                                             lhsT=mtg[:, off + ci, :],
                                             rhs=qtable[:, cw, :],
                                             start=(ci == 0),
                                             stop=(ci == len(cvr) - 1))
                        off += len(cvr)
                    qps4b = scp.tile([P, 4, E], dt.bfloat16, tag="qps4b")
                    nc.scalar.copy(out=qps4b[:], in_=qps4[:])
                    # scores: per-point dot(x, q') over channels
                    prod = prodp.tile([P, 16, CV], dt.bfloat16, tag="prod")
                    nc.vector.tensor_tensor(
                        out=prod[:].rearrange("p (t j) c -> p t j c", t=4),
                        in0=xv.rearrange("p (t j) c -> p t j c", t=4),
                        in1=qps4b[:, :, None, :].to_broadcast([P, 4, 4, E]),
                        op=AL.mult)
                    sc = scp.tile([P, 16], dt.bfloat16, tag="sc")
                    with nc.allow_low_precision(reason="bf16 score reduce"):
                        nc.vector.tensor_reduce(out=sc[:], in_=prod[:],
                                                axis=mybir.AxisListType.X,
                                                op=AL.add)
                    nc.scalar.activation(out=ex[:, gs * 16:(gs + 1) * 16],
                                         in_=sc[:],
                                         func=mybir.ActivationFunctionType.Exp)

                # merged softmax tail + weighted sum at supergroup level
                den = scp.tile([P, TT], dt.float32, tag="den")
                nc.vector.tensor_reduce(
                    out=den[:],
                    in_=ex[:].rearrange("p (t j) -> p t j", t=TT),
                    axis=mybir.AxisListType.X, op=AL.add)
                rec = scp.tile([P, TT], dt.float32, tag="rec")
                nc.vector.reciprocal(out=rec[:], in_=den[:])
                attn = scp.tile([P, NCOL], dt.bfloat16, tag="attn")
                nc.vector.tensor_tensor(
                    out=attn[:].rearrange("p (t j) -> p t j", t=TT),
                    in0=ex[:].rearrange("p (t j) -> p t j", t=TT),
                    in1=rec[:, :, None].to_broadcast([P, TT, 4]),
                    op=AL.mult)
                prod2 = scp.tile([P, NCOL, CV], dt.bfloat16, tag="prod2")
                nc.vector.tensor_tensor(
                    out=prod2[:], in0=xsg[:],
                    in1=attn[:, :, None].to_broadcast([P, NCOL, CV]),
                    op=AL.mult)
                pj = prod2[:].rearrange("p (g t j) c -> p j g t c", g=2, t=4)
                xb = xbar_g[sg][:].rearrange("p (g t) c -> p g t c", g=2)
                nc.vector.tensor_tensor(out=xb, in0=pj[:, 0], in1=pj[:, 1],
                                        op=AL.add)
                nc.vector.tensor_tensor(out=xb, in0=xb, in1=pj[:, 2],
                                        op=AL.add)
                nc.vector.tensor_tensor(out=xb, in0=xb, in1=pj[:, 3],
                                        op=AL.add)

                chunks_done = (sg + 1) * TT
                while done_ct < CT and ct_ready[done_ct] <= chunks_done:
                    emit_scatter(done_ct)
                    done_ct += 1

            build_qtable_to(CTT)
            for ct in range(done_ct, CT):
                emit_scatter(ct)
    nc.compile()
    return nc


def _install_ntff_shim():
    try:
        import antenv.axon_hooks  # noqa
        return
    except ImportError:
        pass
    try:
        from trn_agent_boot.trn_boot import _ntff_profile_via_ctypes
        hook = _ntff_profile_via_ctypes('/opt/axon/libaxon_pjrt.so')
        mod = types.ModuleType("antenv.axon_hooks")
        mod.get_axon_ntff_profile_hook = lambda: hook
        mod.set_axon_ntff_profile_hook = lambda h: None
        import antenv
        antenv.axon_hooks = mod
        sys.modules["antenv.axon_hooks"] = mod
    except Exception:
        pass


def kernel(**inputs):
    v_feat = np.asarray(inputs["v_feat"], np.float32)
    r_feat = np.asarray(inputs["r_feat"], np.float32)
    Wq = np.asarray(inputs["Wq"], np.float32)
    Wk = np.asarray(inputs["Wk"], np.float32)
    Wv = np.asarray(inputs["Wv"], np.float32)
    Wo = np.asarray(inputs["Wo"], np.float32)
    v2p = np.asarray(inputs["v2p_ind"])
    r2p = np.asarray(inputs["r2p_ind"])
    Mv = v_feat.shape[2]
    Mr = r_feat.shape[2]

    plan = _plan(v2p, r2p)
    nc = _build(plan, Mv)

    A16 = (Wq.T @ Wk / np.sqrt(np.float32(E))).astype(BF16)
    WovT16 = np.ascontiguousarray((Wo @ Wv).T).astype(BF16)

    in_maps = []
    vtab_cache = {}
    for c in plan["cores"]:
        arr = _core_arrays(c, plan, v_feat, r_feat, vtab_cache)
        arr["a16"] = A16
        arr["wovT"] = WovT16
        in_maps.append(arr)

    from concourse.bass_utils import run_bass_kernel_spmd
    _install_ntff_shim()
    trace = bool(inputs.get("_trace", False))
    res = run_bass_kernel_spmd(nc, in_maps, core_ids=list(range(8)),
                               trace=trace)
    out = np.zeros((B, CO, Mr), np.float32)
    for ci, c in enumerate(plan["cores"]):
        o = res.results[ci]["out"]
        w = min(c["width"], plan["W_OUT"])
        out[c["b"], :, c["clo"]:c["clo"] + w] = o[:, :w]
    kernel.last_exec_time_ns = res.exec_time_ns
    return out


kernel.last_exec_time_ns = None
